# revision 64
# baseline (speedup 1.0000x reference)
"""Cross-attention transformer block on 8 TRN2 NeuronCores.

Sharding: 8 cores = 4 batches x 2 sequence-halves. Core c handles batch
b = c//2, query tokens [hf*1024, (hf+1)*1024) with hf = c%2. Each core
computes the FULL kv projection for its batch (duplicated across the 2
cores of a batch) so no collectives are needed.

Feature-major layout ([feature, token]) so matmuls contract over the
partition dim with natural weight layouts. Optimizations on top of the
v2 baseline (915us -> ~845us on the fast clock state):
  - q/k/v projections run fp8e4m3 with DoubleRow (256-deep contraction
    per pass, ~1.8x PE); LN outputs are emitted as [128, 2, T] fp8
    pair-tiles so DoubleRow slices them directly. v (vext) stays fp8
    as the U-matmul stationary. W1/W2/Wo remain bf16: fp8 there pushed
    rel err to ~2e-2 (the MLP path has no averaging to wash out
    quantization noise; attention does).
  - LN normalize and RoPE run on bf16 DVE 2x mode; ActE evicts psum
    to bf16 first (ActE is idle in those phases). Rope tables bf16.
  - All reciprocals use the custom-DVE reciprocal_approx_fast (~5x
    over InstReciprocal, which cost 3.3us per [1,512] row). NOTE: the
    custom op only honors base_partition 0 - softmax denominators are
    first copied from psum partition 64 to a partition-0 SBUF row.
  - Emission interleaves LN(x) chunks into the k/v unit stream so the
    LN DVE/ActE work hides under projection PE work; q is emitted last
    so attention (which needs qT[fc] ascending) can start early.
  - Attention: scores as two concurrent K=64 row-tiled matmuls; exp
    [128,1024] per head-pair on ActE (attention is ActE-bound: 284us
    of exp at 1 elem/lane/cycle is the phase floor); U with an
    appended ones-column (M=65) accumulates both U and the softmax
    denominator.
  - Weight DMA prefetch is emitted behind the first input chunk's
    loads; W1/W2 stream with 4-5 deep pools.
"""

import numpy as np
import ml_dtypes

import concourse.bass as bass
import concourse.bacc as bacc
import concourse.mybir as mybir
import concourse.tile as tile
from concourse.bass_utils import run_bass_kernel_spmd

F32 = mybir.dt.float32
F32R = mybir.dt.float32r
BF16 = mybir.dt.bfloat16
FP8 = mybir.dt.float8e4
AF = mybir.ActivationFunctionType
ALU = mybir.AluOpType
DR = mybir.MatmulPerfMode.DoubleRow

B, L, D, H, HD = 4, 2048, 1024, 16, 64
TQ = 1024          # query tokens per core
TK = 2048          # kv tokens per core
HID = 4 * D
NCORES = 8
P = 128
DC = D // P        # 8 feature chunks
KC = TK // P       # 16 kv-token chunks
NHC = HID // P     # 32 hidden chunks
EPS = 1e-5

# vecs[:, i, :] packing indices
(V_BQ, V_BK, V_BO, V_B2, V_BQR, V_BKR) = range(6)

PAIRSWAP_MASK = [i + 1 if i % 2 == 0 else i - 1 for i in range(32)]

_CACHED_NC = None


def _pool(tc, name, bufs, side="left"):
    cm = tc.tile_pool(name=name, bufs=bufs, side=side)
    return cm, cm.__enter__()


def _psum(tc, name):
    cm = tc.tile_pool(name=name, bufs=1, space="PSUM")
    return cm, cm.__enter__()


def build_nc():
    nc = bacc.Bacc("TRN2", debug=False, num_devices=NCORES)

    xT = nc.declare_dram_parameter("xT", [D, TQ], F32R, False).ap()
    ctxT = nc.declare_dram_parameter("ctxT", [D, TK], F32R, False).ap()
    cosq = nc.declare_dram_parameter("cosq", [P, TQ], BF16, False).ap()
    sinq = nc.declare_dram_parameter("sinq", [P, TQ], BF16, False).ap()
    cosk = nc.declare_dram_parameter("cosk", [P, TK], BF16, False).ap()
    sink = nc.declare_dram_parameter("sink", [P, TK], BF16, False).ap()
    # q/k/v weights packed [p, dc, m] = W[dc*128+p, m], fp8 for DoubleRow
    wq = nc.declare_dram_parameter("wq", [P, DC, D], FP8, False).ap()
    wk = nc.declare_dram_parameter("wk", [P, DC, D], FP8, False).ap()
    wv = nc.declare_dram_parameter("wv", [P, DC, D], FP8, False).ap()
    wo = nc.declare_dram_parameter("wo", [D, D], BF16, False).ap()
    # w1 packed [p, hc, dc, j] = W1[dc*128+p, hc*128+j]
    w1 = nc.declare_dram_parameter("w1", [P, NHC, DC, P], BF16, False).ap()
    w2 = nc.declare_dram_parameter("w2", [HID, D], BF16, False).ap()
    vecs_d = nc.declare_dram_parameter("vecs", [P, 6, DC], F32, False).ap()
    b1t_d = nc.declare_dram_parameter("b1t", [P, NHC], F32, False).ap()
    bvrow_d = nc.declare_dram_parameter("bvrow", [1, D], F32, False).ap()
    onesr_d = nc.declare_dram_parameter("onesr", [P, 1], F32R, False).ap()
    outT = nc.declare_dram_parameter("outT", [D, TQ], F32, True).ap()
    import os
    DBG = os.environ.get("KDBG", "0") == "1"
    if DBG:
        d_chat = nc.declare_dram_parameter("d_chat", [P, TK], F32, True).ap()
        d_xhat = nc.declare_dram_parameter("d_xhat", [P, TQ], F32, True).ap()
        d_qT = nc.declare_dram_parameter("d_qT", [P, TQ], F32, True).ap()
        d_kT = nc.declare_dram_parameter("d_kT", [P, TK], F32, True).ap()
        d_attnT = nc.declare_dram_parameter("d_attnT", [P, TQ], F32, True).ap()
        d_x2T = nc.declare_dram_parameter("d_x2T", [P, TQ], F32, True).ap()
        d_pu = nc.declare_dram_parameter("d_pu", [P, 512], F32, True).ap()
        d_rcp = nc.declare_dram_parameter("d_rcp", [1, 512], F32, True).ap()
        d_rb = nc.declare_dram_parameter("d_rb", [1, 512], F32, True).ap()

    with tile.TileContext(nc) as tc:
        const_cm, const = _pool(tc, "const", 1)
        work_cm, work = _pool(tc, "work", 8)       # f32 [128,512] scratch
        stat_cm, stat = _pool(tc, "stat", 4)

        # ---- constants ----
        vecs = const.tile([P, 6, DC], F32, tag="vecs")
        nc.sync.dma_start(vecs[:], vecs_d)
        b1t = const.tile([P, NHC], F32, tag="b1t")
        nc.sync.dma_start(b1t[:], b1t_d)
        bvrow = const.tile([1, D], F32, tag="bvrow")
        nc.sync.dma_start(bvrow[:], bvrow_d)
        bvb = const.tile([P, D], F32, tag="bvb")
        nc.gpsimd.partition_broadcast(bvb[:], bvrow[:])
        onesP = const.tile([P, 1], F32, tag="onesP")
        nc.vector.memset(onesP[:], 1.0)
        onesPr = const.tile([P, 1], F32R, tag="onesPr")
        nc.sync.dma_start(onesPr[:], onesr_d)
        onesPb = const.tile([P, 1], BF16, tag="onesPb")
        nc.vector.memset(onesPb[:], 1.0)
        eps1 = const.tile([1, 1], F32, tag="eps1")
        nc.vector.memset(eps1[:], EPS)

        def scratch(name):
            return work.tile([P, 512], F32, tag="scratch", name=name)

        def gcol(idx, dc):
            return vecs[:, idx, dc : dc + 1]

        def ln_alloc(out_pool, out_tag, nt, paired):
            if paired:
                return [out_pool.tile([P, 2, nt], FP8, tag=out_tag,
                                      name=f"{out_tag}{i}")
                        for i in range(DC // 2)]
            return [out_pool.tile([P, nt], BF16, tag=out_tag,
                                  name=f"{out_tag}{i}") for i in range(DC)]

        def ln_chunk(ps, load_fn, outs, tt, out_tag, src_r, paired):
            """One 512-token LayerNorm chunk (gamma/beta folded host-side)."""
            if True:
                sl = slice(tt * 512, (tt + 1) * 512)
                raw = [load_fn(dc, tt) for dc in range(DC)]
                srcs = [r.bitcast(F32) if src_r else r for r in raw]
                pr_row = ps.tile([P, 512], F32, tag="row", bufs=1,
                                 name=f"lnrow_{out_tag}_{tt}")
                # sum on partition 0, sumsq on partition 32 (same bank)
                for dc in range(DC):
                    sq = work.tile([P, 512], BF16, tag="sq", bufs=2,
                                   name=f"sq_{out_tag}_{tt}_{dc}")
                    nc.scalar.square(sq[:], srcs[dc])
                    nc.tensor.matmul(
                        pr_row[0:1, :],
                        onesPr[:] if src_r else onesPb[:],
                        raw[dc],
                        start=(dc == 0), stop=(dc == DC - 1),
                    )
                    nc.tensor.matmul(
                        pr_row[32:33, :], onesPb[:],
                        sq[:],
                        start=(dc == 0), stop=(dc == DC - 1),
                    )
                st = stat.tile([1, 3, 512], F32, tag="stats", bufs=2,
                               name=f"st_{out_tag}_{tt}")
                mu, var, rs = (st[:, i, :] for i in range(3))
                nc.vector.tensor_scalar_mul(mu, pr_row[0:1, :], 1.0 / D)
                nc.vector.tensor_scalar_mul(rs, pr_row[32:33, :], 1.0 / D)
                nc.vector.tensor_mul(var, mu, mu)
                nc.vector.tensor_sub(var, rs, var)
                # rs <- sqrt(var+eps) then var <- 1/rs (fast approx)
                nc.scalar.activation(rs, var, AF.Sqrt, bias=eps1[:])
                nc.vector.reciprocal_approx_fast(var, rs)
                rs = var
                # bf16 stats rows -> bf16 broadcasts -> bf16 2x normalize
                stb = stat.tile([1, 2, 512], BF16, tag="statsb", bufs=1,
                                name=f"stb_{out_tag}_{tt}")
                mu_b, rs_b = stb[:, 0, :], stb[:, 1, :]
                nc.vector.tensor_copy(mu_b, mu)
                nc.vector.tensor_copy(rs_b, rs)
                mub = work.tile([P, 512], BF16, tag="mub", bufs=2,
                                name=f"mub_{out_tag}_{tt}")
                nc.gpsimd.partition_broadcast(mub[:], mu_b)
                rsb = work.tile([P, 512], BF16, tag="rsb", bufs=2,
                                name=f"rsb_{out_tag}_{tt}")
                nc.gpsimd.partition_broadcast(rsb[:], rs_b)
                for dc in range(DC):
                    # ActE evicts src to bf16 so both DVE ops run 2x mode
                    xb = work.tile([P, 512], BF16, tag="xb", bufs=2,
                                   name=f"xb_{out_tag}_{tt}_{dc}")
                    nc.scalar.activation(xb[:], srcs[dc], AF.Copy)
                    t = work.tile([P, 512], BF16, tag="lnt", bufs=2,
                                  name=f"lnt_{out_tag}_{tt}_{dc}")
                    nc.vector.tensor_sub(t[:], xb[:], mub[:])
                    dst = (outs[dc // 2][:, dc % 2, sl] if paired
                           else outs[dc][:, sl])
                    nc.vector.tensor_mul(dst, t[:], rsb[:])

        def ln_T(ps, load_fn, nt, out_pool, out_tag, src_r, paired=False):
            outs = ln_alloc(out_pool, out_tag, nt, paired)
            for tt in range(nt // 512):
                ln_chunk(ps, load_fn, outs, tt, out_tag, src_r, paired)
            return outs

        def rope_evict(psum, out_ap, cos_t, sin_t, sl, b_idx, b_rot_idx, fc):
            """out = (psum + b)*cosP + (pairswap(psum) + bR)*sinE (bf16).

            ActE evicts the psum to bf16 so the DVE ops run in 2x mode
            (the shuffle stays 1x; its bf16 input at least halves reads).
            """
            pb = work.tile([P, 512], BF16, tag="ropePb", bufs=2,
                           name=f"ropeP_{b_idx}_{fc}_{sl.start}")
            nc.scalar.activation(pb[:], psum[:], AF.Copy)
            sh = work.tile([P, 512], BF16, tag="ropeSh", bufs=2,
                           name=f"ropeS_{b_idx}_{fc}_{sl.start}")
            nc.vector.stream_shuffle(sh[:], pb[:], PAIRSWAP_MASK)
            t = work.tile([P, 512], BF16, tag="ropeT", bufs=2,
                          name=f"ropeA_{b_idx}_{fc}_{sl.start}")
            nc.vector.scalar_tensor_tensor(
                t[:], pb[:], gcol(b_idx, fc), cos_t[:, sl],
                ALU.add, ALU.mult,
            )
            t2 = work.tile([P, 512], BF16, tag="ropeT", bufs=2,
                           name=f"ropeB_{b_idx}_{fc}_{sl.start}")
            nc.vector.scalar_tensor_tensor(
                t2[:], sh[:], gcol(b_rot_idx, fc), sin_t[:, sl],
                ALU.add, ALU.mult,
            )
            nc.vector.tensor_add(out_ap, t[:], t2[:])

        def dram_loader(pool, dram_ap, tag):
            def load(dc, tt):
                t = pool.tile([P, 512], F32R, tag=tag,
                              name=f"{tag}_{dc}_{tt}")
                nc.sync.dma_start(
                    t[:], dram_ap[dc * P : (dc + 1) * P,
                                  tt * 512 : (tt + 1) * 512])
                return t[:]
            return load

        ps1_cm, ps1 = _psum(tc, "ps1")

        def dbg_dump(src_ap, dram_ap, n):
            if not DBG:
                return
            for c in range(n // 512):
                s = work.tile([P, 512], F32, tag="dbgs", bufs=2,
                              name=f"dbg_{dram_ap}_{c}")
                nc.vector.tensor_copy(
                    s[:], src_ap[:, c * 512 : (c + 1) * 512])
                nc.sync.dma_start(
                    dram_ap[:, c * 512 : (c + 1) * 512], s[:])

        # ====== front: LN(ctx), then {LN(x) | k | v | q} interleaved ======
        # weights prefetched up front so DMA hides under LN compute
        w_cm, w_p = _pool(tc, "wqkv", 8, side="right")
        chat_cm, chat_p = _pool(tc, "chat", DC, side="right")
        # left-stack order chosen for LIFO exits:
        # qT/kT/vext (die after attention) below xhat (dies after q),
        # then cin (dies after ctx-LN), then xin (dies after interleave)
        qT_cm, qT_p = _pool(tc, "qT", DC)
        kT_cm, kT_p = _pool(tc, "kT", DC)
        vext_cm, vext_p = _pool(tc, "vext", KC)
        xhat_cm, xhat_p = _pool(tc, "xhat", DC)

        cin_cm, cin_p = _pool(tc, "cin", 12)
        chatT = ln_alloc(chat_p, "chat", TK, paired=True)
        cin_load = dram_loader(cin_p, ctxT, "cin")
        wk_sb = wv_sb = None
        for tt in range(TK // 512):
            ln_chunk(ps1, cin_load, chatT, tt, "chat", True, True)
            if tt == 0:
                # weight prefetch behind the first input chunk's DMAs
                wk_sb = w_p.tile([P, DC, D], FP8, tag="w8", bufs=2,
                                 name="wk8")
                nc.sync.dma_start(wk_sb[:], wk)
                wv_sb = w_p.tile([P, DC, D], FP8, tag="w8", bufs=2,
                                 name="wv8")
                nc.sync.dma_start(wv_sb[:], wv)
        cin_cm.__exit__(None, None, None)

        ropek_cm, ropek_p = _pool(tc, "ropek", 1, side="right")
        cosk_t = ropek_p.tile([P, TK], BF16, tag="cosk")
        nc.sync.dma_start(cosk_t[:], cosk)
        sink_t = ropek_p.tile([P, TK], BF16, tag="sink")
        nc.sync.dma_start(sink_t[:], sink)

        xin_cm, xin_p = _pool(tc, "xin", 8)
        xhatT = ln_alloc(xhat_p, "xhat", TQ, paired=True)
        xin_load = dram_loader(xin_p, xT, "xin")
        kT = [kT_p.tile([P, TK], BF16, tag="kT", name=f"kT{i}")
              for i in range(DC)]
        vext = []
        for kc in range(KC):
            vt = vext_p.tile([P, H, HD + 1], FP8, tag="vext",
                             name=f"vext{kc}")
            nc.vector.memset(vt[:, :, HD : HD + 1], 1.0)
            vext.append(vt)

        def k_unit(fc):
            for tt in range(TK // 512):
                sl = slice(tt * 512, (tt + 1) * 512)
                pm = ps1.tile([P, 512], F32, tag="mm", bufs=2,
                              name=f"pmk_{fc}_{tt}")
                for i in range(DC // 2):
                    nc.tensor.matmul(
                        pm[:],
                        wk_sb[:, 2 * i : 2 * i + 2, fc * P : (fc + 1) * P],
                        chatT[i][:, :, sl],
                        start=(i == 0), stop=(i == DC // 2 - 1),
                        perf_mode=DR,
                    )
                rope_evict(pm, kT[fc][:, sl], cosk_t, sink_t, sl,
                           V_BK, V_BKR, fc)

        def v_unit(kc):
            for f2 in range(2):
                pm = ps1.tile([P, 512], F32, tag="mm", bufs=2,
                              name=f"pmv_{kc}_{f2}")
                for i in range(DC // 2):
                    nc.tensor.matmul(
                        pm[:], chatT[i][:, :, kc * P : (kc + 1) * P],
                        wv_sb[:, 2 * i : 2 * i + 2,
                              f2 * 512 : (f2 + 1) * 512],
                        start=(i == 0), stop=(i == DC // 2 - 1),
                        perf_mode=DR,
                    )
                nc.vector.tensor_add(
                    vext[kc][:, f2 * 8 : (f2 + 1) * 8, 0:HD],
                    pm[:].rearrange("p (h d) -> p h d", d=HD),
                    bvb[:, f2 * 512 : (f2 + 1) * 512].rearrange(
                        "p (h d) -> p h d", d=HD),
                )

        def q_unit(fc):
            for tt in range(TQ // 512):
                sl = slice(tt * 512, (tt + 1) * 512)
                pm = ps1.tile([P, 512], F32, tag="mm", bufs=2,
                              name=f"pmq_{fc}_{tt}")
                for i in range(DC // 2):
                    nc.tensor.matmul(
                        pm[:],
                        wq_sb[:, 2 * i : 2 * i + 2, fc * P : (fc + 1) * P],
                        xhatT[i][:, :, sl],
                        start=(i == 0), stop=(i == DC // 2 - 1),
                        perf_mode=DR,
                    )
                rope_evict(pm, qT[fc][:, sl], cosq_t, sinq_t, sl,
                           V_BQ, V_BQR, fc)

        # interleave: PE-heavy k/v units hide LN(x)/rope-k DVE+ActE work
        for fc in range(DC):
            k_unit(fc)
            v_unit(2 * fc)
            v_unit(2 * fc + 1)
            if fc < TQ // 512:
                ln_chunk(ps1, xin_load, xhatT, fc, "xhat", True, True)
        xin_cm.__exit__(None, None, None)
        ropek_cm.__exit__(None, None, None)
        chat_cm.__exit__(None, None, None)

        # q last (xin/ropek space freed); attention follows per-fc
        wqp_cm, wqp_p = _pool(tc, "wqp", 1, side="right")
        wq_sb = wqp_p.tile([P, DC, D], FP8, tag="wq8", name="wq8")
        nc.sync.dma_start(wq_sb[:], wq)
        ropeq_cm, ropeq_p = _pool(tc, "ropeq", 1, side="right")
        cosq_t = ropeq_p.tile([P, TQ], BF16, tag="cosq")
        nc.sync.dma_start(cosq_t[:], cosq)
        sinq_t = ropeq_p.tile([P, TQ], BF16, tag="sinq")
        nc.sync.dma_start(sinq_t[:], sinq)
        qT = [qT_p.tile([P, TQ], BF16, tag="qT", name=f"qT{i}")
              for i in range(DC)]
        for fc in range(DC):
            q_unit(fc)
        ropeq_cm.__exit__(None, None, None)
        wqp_cm.__exit__(None, None, None)
        xhat_cm.__exit__(None, None, None)
        w_cm.__exit__(None, None, None)
        ps1_cm.__exit__(None, None, None)

        # ================= phase 5: attention =================
        # Per (tt, head-pair): 16 kc steps. Each kc: two concurrent K=64
        # score matmuls (PE row tiles 0/64) into a 2-bank psum pair-tile,
        # one [128,1024] exp, two U accumulations (M=65, ones-column
        # denominator). Head pairs processed two at a time so softmax
        # reciprocals batch 4 heads per DVE call.
        ps5_cm, ps5 = _psum(tc, "ps5")
        # wo prefetched here so its DMA hides under attention
        wo_cm, wo_p = _pool(tc, "wo", DC, side="right")
        wo_sb = []
        for dc in range(DC):
            wt = wo_p.tile([P, D], BF16, tag="wo", name=f"wo{dc}")
            nc.sync.dma_start(wt[:], wo[dc * P : (dc + 1) * P, :])
            wo_sb.append(wt)
        attnT_cm, attnT_p = _pool(tc, "attnT", DC, side="right")
        e_cm, e_p = _pool(tc, "epool", 4)
        attnT = [attnT_p.tile([P, TQ], BF16, tag="attnT", name=f"attnT{i}")
                 for i in range(DC)]
        for tt in range(TQ // 512):
            sl = slice(tt * 512, (tt + 1) * 512)
            for hp2 in range(4):       # pairs of head-pairs
                pus = []
                for i in range(2):     # head pair index within group
                    fc = hp2 * 2 + i
                    pu = ps5.tile([P, 2, 512], F32, tag="u", bufs=2,
                                  name=f"pu_{tt}_{fc}")
                    pus.append(pu)
                    for kc in range(KC):
                        psc = ps5.tile([P, 2, 512], F32, tag="sc", bufs=2,
                                       name=f"psc_{tt}_{fc}_{kc}")
                        for j in range(2):   # head row-halves, concurrent
                            hb = j * HD
                            nc.tensor.matmul(
                                psc[:, j, :],
                                kT[fc][hb : hb + HD, kc * P : (kc + 1) * P],
                                qT[fc][hb : hb + HD, sl],
                                start=True, stop=True,
                            )
                        e = e_p.tile([P, 2, 512], BF16, tag="e",
                                     name=f"e_{tt}_{fc}_{kc}")
                        nc.scalar.activation(e[:], psc[:], AF.Exp, scale=0.125)
                        for j in range(2):
                            nc.tensor.matmul(
                                pu[0 : HD + 1, j, :],
                                vext[kc][:, fc * 2 + j, :],
                                e[:, j, :],
                                start=(kc == 0), stop=(kc == KC - 1),
                            )
                # softmax epilogue: copy den row to a partition-0 SBUF tile
                # (the custom-DVE reciprocal only honors base_partition 0),
                # then one fast-approx reciprocal for both heads
                for i in range(2):
                    fc = hp2 * 2 + i
                    den0 = stat.tile([1, 2, 512], F32, tag="den0", bufs=1,
                                     name=f"den0_{tt}_{fc}")
                    nc.vector.tensor_copy(
                        den0[:], pus[i][HD : HD + 1, :, :])
                    rcp = stat.tile([1, 2, 512], F32, tag="rcp", bufs=2,
                                    name=f"rcp_{tt}_{fc}")
                    nc.vector.reciprocal_approx_fast(rcp[:], den0[:])
                    for j in range(2):
                        hb = j * HD
                        rb = work.tile([HD, 512], F32, tag="rb", bufs=2,
                                       name=f"rb_{tt}_{fc}_{j}")
                        nc.gpsimd.partition_broadcast(rb[:], rcp[:, j, :])
                        nc.vector.tensor_mul(
                            attnT[fc][hb : hb + HD, sl],
                            pus[i][0:HD, j, :], rb[:])
        if DBG:
            dbg_dump(attnT[0][:], d_attnT, TQ)
        e_cm.__exit__(None, None, None)
        vext_cm.__exit__(None, None, None)
        kT_cm.__exit__(None, None, None)
        qT_cm.__exit__(None, None, None)
        ps5_cm.__exit__(None, None, None)

        # ================= phase 6: x2^T = Wo^T attn + x^T + bo ========
        ps6_cm, ps6 = _psum(tc, "ps6")
        x2_cm, x2_p = _pool(tc, "x2", DC)
        xin6_cm, xin6_p = _pool(tc, "xin6", 4, side="right")
        xhat2_cm, xhat2_p = _pool(tc, "xhat2", DC, side="right")
        h1_cm, h1_p = _pool(tc, "h1", 2 * NHC)
        w1_cm, w1_p = _pool(tc, "w1s", 4)
        x2T = [x2_p.tile([P, TQ], BF16, tag="x2", name=f"x2T{i}")
               for i in range(DC)]
        xhat2T = ln_alloc(xhat2_p, "xhat2", TQ, False)
        h1 = {}

        def outproj_unit(tt, fc):
            sl = slice(tt * 512, (tt + 1) * 512)
            xres = xin6_p.tile([P, 512], F32R, tag="xin6",
                               name=f"xres_{fc}_{tt}")
            nc.sync.dma_start(xres[:], xT[fc * P : (fc + 1) * P, sl])
            pm = ps6.tile([P, 512], F32, tag="mm", bufs=2,
                          name=f"pmo_{fc}_{tt}")
            for dc in range(DC):
                nc.tensor.matmul(
                    pm[:], wo_sb[dc][:, fc * P : (fc + 1) * P],
                    attnT[dc][:, sl], start=(dc == 0), stop=(dc == DC - 1),
                )
            nc.vector.scalar_tensor_tensor(
                x2T[fc][:, sl], pm[:], gcol(V_BO, fc),
                xres[:].bitcast(F32),
                ALU.add, ALU.add,
            )

        def w1_unit(tt, hc):
            sl = slice(tt * 512, (tt + 1) * 512)
            w1t = w1_p.tile([P, DC, P], BF16, tag="w1",
                            name=f"w1_{tt}_{hc}")
            nc.sync.dma_start(w1t[:], w1[:, hc, :, :])
            ph = ps6.tile([P, 512], F32, tag="mm", bufs=2,
                          name=f"ph1_{tt}_{hc}")
            for dc in range(DC):
                nc.tensor.matmul(
                    ph[:], w1t[:, dc, :],
                    xhat2T[dc][:, sl],
                    start=(dc == 0), stop=(dc == DC - 1),
                )
            ht = h1_p.tile([P, 512], BF16, tag="h1", name=f"h1_{tt}_{hc}")
            nc.scalar.activation(ht[:], ph[:], AF.Gelu,
                                 bias=b1t[:, hc : hc + 1])
            h1[tt, hc] = ht

        x2_load = lambda dc, tt: x2T[dc][:, tt * 512 : (tt + 1) * 512]
        for fc in range(DC):
            outproj_unit(0, fc)
        ln_chunk(ps6, x2_load, xhat2T, 0, "xhat2", False, False)
        # W1(tt0) PE work hides outproj(tt1)/LN2(tt1) DVE+ActE work
        for fc in range(DC):
            outproj_unit(1, fc)
            for hc in range(4 * fc, 4 * fc + 4):
                w1_unit(0, hc)
        ln_chunk(ps6, x2_load, xhat2T, 1, "xhat2", False, False)
        for hc in range(NHC):
            w1_unit(1, hc)
        if DBG:
            dbg_dump(x2T[0][:], d_x2T, TQ)
        w1_cm.__exit__(None, None, None)
        xhat2_cm.__exit__(None, None, None)
        xin6_cm.__exit__(None, None, None)
        attnT_cm.__exit__(None, None, None)
        wo_cm.__exit__(None, None, None)
        ps6_cm.__exit__(None, None, None)

        # ================= phase 8b: MLP down-proj, 8-bank pass ========
        ps8_cm, ps8 = _psum(tc, "ps8")
        w2_cm, w2_p = _pool(tc, "w2s", 5)
        out_cm, out_p = _pool(tc, "ostage", 4)
        for tt in range(TQ // 512):
            sl = slice(tt * 512, (tt + 1) * 512)
            pms = ps8.tile([P, DC, 512], F32, tag="mlp8", bufs=1,
                           name=f"pmh2_{tt}")
            for hc in range(NHC):
                w2t = w2_p.tile([P, D], BF16, tag="w2",
                                name=f"w2_{tt}_{hc}")
                nc.sync.dma_start(w2t[:], w2[hc * P : (hc + 1) * P, :])
                for fc in range(DC):
                    nc.tensor.matmul(
                        pms[:, fc, :], w2t[:, fc * P : (fc + 1) * P],
                        h1[tt, hc][:], start=(hc == 0), stop=(hc == NHC - 1),
                    )
            for fc in range(DC):
                ot = out_p.tile([P, 512], F32, tag="ostage",
                                name=f"ot_{tt}_{fc}")
                nc.vector.scalar_tensor_tensor(
                    ot[:], pms[:, fc, :], gcol(V_B2, fc), x2T[fc][:, sl],
                    ALU.add, ALU.add,
                )
                nc.sync.dma_start(outT[fc * P : (fc + 1) * P, sl], ot[:])

        out_cm.__exit__(None, None, None)
        w2_cm.__exit__(None, None, None)
        h1_cm.__exit__(None, None, None)
        x2_cm.__exit__(None, None, None)
        ps8_cm.__exit__(None, None, None)
        stat_cm.__exit__(None, None, None)
        work_cm.__exit__(None, None, None)
        const_cm.__exit__(None, None, None)

    nc.compile()
    return nc


# old feature index (within a 64-dim head block) at each new position:
# pairs (j, j+32) become adjacent (2j, 2j+1)
OLD_OF_NEW = np.array([j // 2 if j % 2 == 0 else j // 2 + 32
                       for j in range(HD)])


def _perm_cols(a):
    """Permute the last dim (64-multiple) per 64-feature head block."""
    a = np.asarray(a, np.float32)
    shp = a.shape
    nb = shp[-1] // HD
    a = a.reshape(shp[:-1] + (nb, HD))
    a = a[..., OLD_OF_NEW]
    return a.reshape(shp)


def _pairswap(a):
    """Swap even/odd positions of the last dim."""
    a = np.asarray(a, np.float32)
    shp = a.shape
    a = a.reshape(shp[:-1] + (shp[-1] // 2, 2))
    a = a[..., ::-1]
    return np.ascontiguousarray(a.reshape(shp))


def _col8(v):
    return np.ascontiguousarray(
        np.asarray(v, np.float32).reshape(DC, P).T.astype(np.float32))


def make_in_maps(inputs):
    x = np.asarray(inputs["x"], np.float32)
    context = np.asarray(inputs["context"], np.float32)
    cos = np.asarray(inputs["rope_cos"], np.float32).reshape(L, HD)
    sin = np.asarray(inputs["rope_sin"], np.float32).reshape(L, HD)

    bf = lambda a: np.ascontiguousarray(np.asarray(a, np.float32)).astype(
        ml_dtypes.bfloat16)
    Wq = np.asarray(inputs["Wq"], np.float32)
    Wkv = np.asarray(inputs["Wkv"], np.float32)
    W1 = np.asarray(inputs["W1"], np.float32)
    g_q = np.asarray(inputs["g_q"], np.float32)
    be_q = np.asarray(inputs["be_q"], np.float32)
    g_kv = np.asarray(inputs["g_kv"], np.float32)
    be_kv = np.asarray(inputs["be_kv"], np.float32)
    g_ffn = np.asarray(inputs["g_ffn"], np.float32)
    be_ffn = np.asarray(inputs["be_ffn"], np.float32)

    # fold LN gamma/beta into weights/biases
    Wq_f = g_q[:, None] * Wq
    bq_f = be_q @ Wq + np.asarray(inputs["bq"], np.float32)
    Wk_f = g_kv[:, None] * Wkv[:, :D]
    bk_f = be_kv @ Wkv[:, :D] + np.asarray(inputs["bkv"], np.float32)[:D]
    Wv_f = g_kv[:, None] * Wkv[:, D:]
    bv_f = be_kv @ Wkv[:, D:] + np.asarray(inputs["bkv"], np.float32)[D:]
    W1_f = g_ffn[:, None] * W1
    b1_f = be_ffn @ W1 + np.asarray(inputs["b1"], np.float32)

    # rope pair permutation on q/k output features
    Wq_p = _perm_cols(Wq_f)
    bq_p = _perm_cols(bq_f)
    Wk_p = _perm_cols(Wk_f)
    bk_p = _perm_cols(bk_f)

    # q/k/v weights: fp8e4m3 packed [p, dc, m] = W[dc*128+p, m]
    f8 = lambda a: np.ascontiguousarray(
        np.asarray(a, np.float32).reshape(DC, P, D).transpose(1, 0, 2)
    ).astype(ml_dtypes.float8_e4m3fn)
    wq_b = f8(Wq_p)
    wk_b = f8(Wk_p)
    wv_b = f8(Wv_f)
    wo_b = bf(inputs["Wo"])
    # w1 packed [p, hc, dc, j] = W1[dc*128+p, hc*128+j]
    w1_b = bf(np.ascontiguousarray(
        W1_f.reshape(DC, P, NHC, P).transpose(1, 2, 0, 3)))
    w2_b = bf(inputs["W2"])

    vecs = np.stack(
        [_col8(bq_p), _col8(bk_p),
         _col8(inputs["bo"]), _col8(inputs["b2"]),
         _col8(_pairswap(bq_p)), _col8(_pairswap(bk_p))],
        axis=1,
    )  # [128, 6, 8]
    vecs = np.ascontiguousarray(vecs)
    b1t = np.ascontiguousarray(b1_f.reshape(NHC, P).T)
    bvrow = np.ascontiguousarray(bv_f.reshape(1, D))

    # rope tables in permuted feature space:
    # cosP[n] = cos[old_of_new[n]]; sinE[2j] = -sin[j], sinE[2j+1] = sin[j+32]
    cosP = cos[:, OLD_OF_NEW]                        # [L, 64]
    sinP = sin[:, OLD_OF_NEW]
    sinE = sinP.copy()
    sinE[:, 0::2] = -sinE[:, 0::2]
    cosT = cosP.T                                    # [64, L]
    sinT = sinE.T
    cosk_full = np.ascontiguousarray(np.concatenate([cosT, cosT], 0)).astype(
        ml_dtypes.bfloat16)
    sink_full = np.ascontiguousarray(np.concatenate([sinT, sinT], 0)).astype(
        ml_dtypes.bfloat16)

    in_maps = []
    for c in range(NCORES):
        b, hf = c // 2, c % 2
        tsl = slice(hf * TQ, (hf + 1) * TQ)
        in_maps.append({
            "xT": np.ascontiguousarray(x[b, tsl, :].T),
            "ctxT": np.ascontiguousarray(context[b].T),
            "cosq": np.ascontiguousarray(cosk_full[:, tsl]),
            "sinq": np.ascontiguousarray(sink_full[:, tsl]),
            "cosk": cosk_full,
            "sink": sink_full,
            "wq": wq_b, "wk": wk_b, "wv": wv_b, "wo": wo_b,
            "w1": w1_b, "w2": w2_b,
            "vecs": vecs, "b1t": b1t, "bvrow": bvrow,
            "onesr": np.ones((P, 1), np.float32),
        })
    return in_maps


def kernel(**inputs) -> np.ndarray:
    global _CACHED_NC
    if _CACHED_NC is None:
        _CACHED_NC = build_nc()
    nc = _CACHED_NC
    in_maps = make_in_maps(inputs)
    res = run_bass_kernel_spmd(nc, in_maps, core_ids=list(range(NCORES)))
    out = np.empty((B, L, D), np.float32)
    for c in range(NCORES):
        b, hf = c // 2, c % 2
        out[b, hf * TQ : (hf + 1) * TQ, :] = res.results[c]["outT"].T
    return out



# revision 66
# speedup vs baseline: 1.0231x; 1.0231x over previous
"""Cross-attention transformer block on 8 TRN2 NeuronCores.

Sharding: 8 cores = 4 batches x 2 sequence-halves. Core c handles batch
b = c//2, query tokens [hf*1024, (hf+1)*1024) with hf = c%2. Each core
computes the FULL kv projection for its batch (duplicated across the 2
cores of a batch) so no collectives are needed.

Feature-major layout ([feature, token]) so matmuls contract over the
partition dim with natural weight layouts. Optimizations on top of the
v2 baseline (915us -> ~845us on the fast clock state):
  - q/k/v projections run fp8e4m3 with DoubleRow (256-deep contraction
    per pass, ~1.8x PE); LN outputs are emitted as [128, 2, T] fp8
    pair-tiles so DoubleRow slices them directly. v (vext) stays fp8
    as the U-matmul stationary. W1/W2/Wo remain bf16: fp8 there pushed
    rel err to ~2e-2 (the MLP path has no averaging to wash out
    quantization noise; attention does).
  - LN normalize and RoPE run on bf16 DVE 2x mode; ActE evicts psum
    to bf16 first (ActE is idle in those phases). Rope tables bf16.
  - All reciprocals use the custom-DVE reciprocal_approx_fast (~5x
    over InstReciprocal, which cost 3.3us per [1,512] row). NOTE: the
    custom op only honors base_partition 0 - softmax denominators are
    first copied from psum partition 64 to a partition-0 SBUF row.
  - Emission interleaves LN(x) chunks into the k/v unit stream so the
    LN DVE/ActE work hides under projection PE work; q is emitted last
    so attention (which needs qT[fc] ascending) can start early.
  - Attention: scores as two concurrent K=64 row-tiled matmuls; exp
    [128,1024] per head-pair on ActE (attention is ActE-bound: 284us
    of exp at 1 elem/lane/cycle is the phase floor); U with an
    appended ones-column (M=65) accumulates both U and the softmax
    denominator.
  - Weight DMA prefetch is emitted behind the first input chunk's
    loads; W1/W2 stream with 4-5 deep pools.
"""

import numpy as np
import ml_dtypes

import concourse.bass as bass
import concourse.bacc as bacc
import concourse.mybir as mybir
import concourse.tile as tile
from concourse.bass_utils import run_bass_kernel_spmd

F32 = mybir.dt.float32
F32R = mybir.dt.float32r
BF16 = mybir.dt.bfloat16
FP8 = mybir.dt.float8e4
AF = mybir.ActivationFunctionType
ALU = mybir.AluOpType
DR = mybir.MatmulPerfMode.DoubleRow

B, L, D, H, HD = 4, 2048, 1024, 16, 64
TQ = 1024          # query tokens per core
TK = 2048          # kv tokens per core
HID = 4 * D
NCORES = 8
P = 128
DC = D // P        # 8 feature chunks
KC = TK // P       # 16 kv-token chunks
NHC = HID // P     # 32 hidden chunks
EPS = 1e-5

# vecs[:, i, :] packing indices
(V_BQ, V_BK, V_BO, V_B2, V_BQR, V_BKR) = range(6)

PAIRSWAP_MASK = [i + 1 if i % 2 == 0 else i - 1 for i in range(32)]

_CACHED_NC = None


def _pool(tc, name, bufs, side="left"):
    cm = tc.tile_pool(name=name, bufs=bufs, side=side)
    return cm, cm.__enter__()


def _psum(tc, name):
    cm = tc.tile_pool(name=name, bufs=1, space="PSUM")
    return cm, cm.__enter__()


def build_nc():
    nc = bacc.Bacc("TRN2", debug=False, num_devices=NCORES)

    xT = nc.declare_dram_parameter("xT", [D, TQ], F32R, False).ap()
    ctxT = nc.declare_dram_parameter("ctxT", [D, TK], F32R, False).ap()
    cosq = nc.declare_dram_parameter("cosq", [P, TQ], BF16, False).ap()
    sinq = nc.declare_dram_parameter("sinq", [P, TQ], BF16, False).ap()
    cosk = nc.declare_dram_parameter("cosk", [P, TK], BF16, False).ap()
    sink = nc.declare_dram_parameter("sink", [P, TK], BF16, False).ap()
    # q/k/v weights packed [p, dc, m] = W[dc*128+p, m], fp8 for DoubleRow
    wq = nc.declare_dram_parameter("wq", [P, DC, D], FP8, False).ap()
    wk = nc.declare_dram_parameter("wk", [P, DC, D], FP8, False).ap()
    wv = nc.declare_dram_parameter("wv", [P, DC, D], FP8, False).ap()
    wo = nc.declare_dram_parameter("wo", [D, D], BF16, False).ap()
    # w1 packed [p, hc, dc, j] = W1[dc*128+p, hc*128+j]
    w1 = nc.declare_dram_parameter("w1", [P, NHC, DC, P], BF16, False).ap()
    w2 = nc.declare_dram_parameter("w2", [HID, D], BF16, False).ap()
    vecs_d = nc.declare_dram_parameter("vecs", [P, 6, DC], F32, False).ap()
    b1t_d = nc.declare_dram_parameter("b1t", [P, NHC], F32, False).ap()
    bvrow_d = nc.declare_dram_parameter("bvrow", [1, D], F32, False).ap()
    onesr_d = nc.declare_dram_parameter("onesr", [P, 1], F32R, False).ap()
    outT = nc.declare_dram_parameter("outT", [D, TQ], F32, True).ap()
    import os
    DBG = os.environ.get("KDBG", "0") == "1"
    if DBG:
        d_chat = nc.declare_dram_parameter("d_chat", [P, TK], F32, True).ap()
        d_xhat = nc.declare_dram_parameter("d_xhat", [P, TQ], F32, True).ap()
        d_qT = nc.declare_dram_parameter("d_qT", [P, TQ], F32, True).ap()
        d_kT = nc.declare_dram_parameter("d_kT", [P, TK], F32, True).ap()
        d_attnT = nc.declare_dram_parameter("d_attnT", [P, TQ], F32, True).ap()
        d_x2T = nc.declare_dram_parameter("d_x2T", [P, TQ], F32, True).ap()
        d_pu = nc.declare_dram_parameter("d_pu", [P, 512], F32, True).ap()
        d_rcp = nc.declare_dram_parameter("d_rcp", [1, 512], F32, True).ap()
        d_rb = nc.declare_dram_parameter("d_rb", [1, 512], F32, True).ap()

    with tile.TileContext(nc) as tc:
        const_cm, const = _pool(tc, "const", 1)
        work_cm, work = _pool(tc, "work", 8)       # f32 [128,512] scratch
        stat_cm, stat = _pool(tc, "stat", 4)

        # ---- constants ----
        vecs = const.tile([P, 6, DC], F32, tag="vecs")
        nc.sync.dma_start(vecs[:], vecs_d)
        b1t = const.tile([P, NHC], F32, tag="b1t")
        nc.sync.dma_start(b1t[:], b1t_d)
        bvrow = const.tile([1, D], F32, tag="bvrow")
        nc.sync.dma_start(bvrow[:], bvrow_d)
        bvb = const.tile([P, D], F32, tag="bvb")
        nc.gpsimd.partition_broadcast(bvb[:], bvrow[:])
        onesP = const.tile([P, 1], F32, tag="onesP")
        nc.vector.memset(onesP[:], 1.0)
        onesPr = const.tile([P, 1], F32R, tag="onesPr")
        nc.sync.dma_start(onesPr[:], onesr_d)
        onesPb = const.tile([P, 1], BF16, tag="onesPb")
        nc.vector.memset(onesPb[:], 1.0)
        eps1 = const.tile([1, 1], F32, tag="eps1")
        nc.vector.memset(eps1[:], EPS)

        def scratch(name):
            return work.tile([P, 512], F32, tag="scratch", name=name)

        def gcol(idx, dc):
            return vecs[:, idx, dc : dc + 1]

        def ln_alloc(out_pool, out_tag, nt, paired):
            if paired:
                return [out_pool.tile([P, 2, nt], FP8, tag=out_tag,
                                      name=f"{out_tag}{i}")
                        for i in range(DC // 2)]
            return [out_pool.tile([P, nt], BF16, tag=out_tag,
                                  name=f"{out_tag}{i}") for i in range(DC)]

        def ln_chunk(ps, load_fn, outs, tt, out_tag, src_r, paired):
            """One 512-token LayerNorm chunk (gamma/beta folded host-side)."""
            if True:
                sl = slice(tt * 512, (tt + 1) * 512)
                raw = [load_fn(dc, tt) for dc in range(DC)]
                srcs = [r.bitcast(F32) if src_r else r for r in raw]
                pr_row = ps.tile([P, 512], F32, tag="row", bufs=1,
                                 name=f"lnrow_{out_tag}_{tt}")
                # sum on partition 0, sumsq on partition 32 (same bank)
                for dc in range(DC):
                    sq = work.tile([P, 512], BF16, tag="sq", bufs=2,
                                   name=f"sq_{out_tag}_{tt}_{dc}")
                    nc.scalar.square(sq[:], srcs[dc])
                    nc.tensor.matmul(
                        pr_row[0:1, :],
                        onesPr[:] if src_r else onesPb[:],
                        raw[dc],
                        start=(dc == 0), stop=(dc == DC - 1),
                    )
                    nc.tensor.matmul(
                        pr_row[32:33, :], onesPb[:],
                        sq[:],
                        start=(dc == 0), stop=(dc == DC - 1),
                    )
                st = stat.tile([1, 3, 512], F32, tag="stats", bufs=2,
                               name=f"st_{out_tag}_{tt}")
                mu, var, rs = (st[:, i, :] for i in range(3))
                nc.vector.tensor_scalar_mul(mu, pr_row[0:1, :], 1.0 / D)
                nc.vector.tensor_scalar_mul(rs, pr_row[32:33, :], 1.0 / D)
                nc.vector.tensor_mul(var, mu, mu)
                nc.vector.tensor_sub(var, rs, var)
                # rs <- sqrt(var+eps) then var <- 1/rs (fast approx)
                nc.scalar.activation(rs, var, AF.Sqrt, bias=eps1[:])
                nc.vector.reciprocal_approx_fast(var, rs)
                rs = var
                # bf16 stats rows -> bf16 broadcasts -> bf16 2x normalize
                stb = stat.tile([1, 2, 512], BF16, tag="statsb", bufs=1,
                                name=f"stb_{out_tag}_{tt}")
                mu_b, rs_b = stb[:, 0, :], stb[:, 1, :]
                nc.vector.tensor_copy(mu_b, mu)
                nc.vector.tensor_copy(rs_b, rs)
                mub = work.tile([P, 512], BF16, tag="mub", bufs=2,
                                name=f"mub_{out_tag}_{tt}")
                nc.gpsimd.partition_broadcast(mub[:], mu_b)
                rsb = work.tile([P, 512], BF16, tag="rsb", bufs=2,
                                name=f"rsb_{out_tag}_{tt}")
                nc.gpsimd.partition_broadcast(rsb[:], rs_b)
                for dc in range(DC):
                    # ActE evicts src to bf16 so both DVE ops run 2x mode
                    xb = work.tile([P, 512], BF16, tag="xb", bufs=2,
                                   name=f"xb_{out_tag}_{tt}_{dc}")
                    nc.scalar.activation(xb[:], srcs[dc], AF.Copy)
                    t = work.tile([P, 512], BF16, tag="lnt", bufs=2,
                                  name=f"lnt_{out_tag}_{tt}_{dc}")
                    nc.vector.tensor_sub(t[:], xb[:], mub[:])
                    dst = (outs[dc // 2][:, dc % 2, sl] if paired
                           else outs[dc][:, sl])
                    nc.vector.tensor_mul(dst, t[:], rsb[:])

        def ln_T(ps, load_fn, nt, out_pool, out_tag, src_r, paired=False):
            outs = ln_alloc(out_pool, out_tag, nt, paired)
            for tt in range(nt // 512):
                ln_chunk(ps, load_fn, outs, tt, out_tag, src_r, paired)
            return outs

        def rope_evict(psum, out_ap, cos_t, sin_t, sl, b_idx, b_rot_idx, fc):
            """out = (psum + b)*cosP + pairswap(psum + b)*sinE (bf16).

            bR = pairswap(b), so applying b on the ActE eviction (bias is
            per-partition there, free) makes the shuffled copy carry bR
            automatically; both DVE multiplies then run as bf16 2x TT
            (STT has no 2x uop and costs ~745ns vs 345ns for TT).
            """
            pb = work.tile([P, 512], BF16, tag="ropePb", bufs=2,
                           name=f"ropeP_{b_idx}_{fc}_{sl.start}")
            nc.scalar.activation(pb[:], psum[:], AF.Identity,
                                 bias=gcol(b_idx, fc))
            sh = work.tile([P, 512], BF16, tag="ropeSh", bufs=2,
                           name=f"ropeS_{b_idx}_{fc}_{sl.start}")
            nc.vector.stream_shuffle(sh[:], pb[:], PAIRSWAP_MASK)
            t = work.tile([P, 512], BF16, tag="ropeT", bufs=2,
                          name=f"ropeA_{b_idx}_{fc}_{sl.start}")
            nc.vector.tensor_mul(t[:], pb[:], cos_t[:, sl])
            t2 = work.tile([P, 512], BF16, tag="ropeT", bufs=2,
                           name=f"ropeB_{b_idx}_{fc}_{sl.start}")
            nc.vector.tensor_mul(t2[:], sh[:], sin_t[:, sl])
            nc.vector.tensor_add(out_ap, t[:], t2[:])

        def dram_loader(pool, dram_ap, tag):
            def load(dc, tt):
                t = pool.tile([P, 512], F32R, tag=tag,
                              name=f"{tag}_{dc}_{tt}")
                nc.sync.dma_start(
                    t[:], dram_ap[dc * P : (dc + 1) * P,
                                  tt * 512 : (tt + 1) * 512])
                return t[:]
            return load

        ps1_cm, ps1 = _psum(tc, "ps1")

        def dbg_dump(src_ap, dram_ap, n):
            if not DBG:
                return
            for c in range(n // 512):
                s = work.tile([P, 512], F32, tag="dbgs", bufs=2,
                              name=f"dbg_{dram_ap}_{c}")
                nc.vector.tensor_copy(
                    s[:], src_ap[:, c * 512 : (c + 1) * 512])
                nc.sync.dma_start(
                    dram_ap[:, c * 512 : (c + 1) * 512], s[:])

        # ====== front: LN(ctx), then {LN(x) | k | v | q} interleaved ======
        # weights prefetched up front so DMA hides under LN compute
        w_cm, w_p = _pool(tc, "wqkv", 8, side="right")
        chat_cm, chat_p = _pool(tc, "chat", DC, side="right")
        # left-stack order chosen for LIFO exits:
        # qT/kT/vext (die after attention) below xhat (dies after q),
        # then cin (dies after ctx-LN), then xin (dies after interleave)
        qT_cm, qT_p = _pool(tc, "qT", DC)
        kT_cm, kT_p = _pool(tc, "kT", DC)
        vext_cm, vext_p = _pool(tc, "vext", KC)
        xhat_cm, xhat_p = _pool(tc, "xhat", DC)

        cin_cm, cin_p = _pool(tc, "cin", 12)
        chatT = ln_alloc(chat_p, "chat", TK, paired=True)
        cin_load = dram_loader(cin_p, ctxT, "cin")
        wk_sb = wv_sb = None
        for tt in range(TK // 512):
            ln_chunk(ps1, cin_load, chatT, tt, "chat", True, True)
            if tt == 0:
                # weight prefetch behind the first input chunk's DMAs
                wk_sb = w_p.tile([P, DC, D], FP8, tag="w8", bufs=2,
                                 name="wk8")
                nc.sync.dma_start(wk_sb[:], wk)
                wv_sb = w_p.tile([P, DC, D], FP8, tag="w8", bufs=2,
                                 name="wv8")
                nc.sync.dma_start(wv_sb[:], wv)
        cin_cm.__exit__(None, None, None)

        ropek_cm, ropek_p = _pool(tc, "ropek", 1, side="right")
        cosk_t = ropek_p.tile([P, TK], BF16, tag="cosk")
        nc.sync.dma_start(cosk_t[:], cosk)
        sink_t = ropek_p.tile([P, TK], BF16, tag="sink")
        nc.sync.dma_start(sink_t[:], sink)

        xin_cm, xin_p = _pool(tc, "xin", 8)
        xhatT = ln_alloc(xhat_p, "xhat", TQ, paired=True)
        xin_load = dram_loader(xin_p, xT, "xin")
        kT = [kT_p.tile([P, TK], BF16, tag="kT", name=f"kT{i}")
              for i in range(DC)]
        vext = []
        for kc in range(KC):
            vt = vext_p.tile([P, H, HD + 1], FP8, tag="vext",
                             name=f"vext{kc}")
            nc.vector.memset(vt[:, :, HD : HD + 1], 1.0)
            vext.append(vt)

        def k_unit(fc):
            for tt in range(TK // 512):
                sl = slice(tt * 512, (tt + 1) * 512)
                pm = ps1.tile([P, 512], F32, tag="mm", bufs=2,
                              name=f"pmk_{fc}_{tt}")
                for i in range(DC // 2):
                    nc.tensor.matmul(
                        pm[:],
                        wk_sb[:, 2 * i : 2 * i + 2, fc * P : (fc + 1) * P],
                        chatT[i][:, :, sl],
                        start=(i == 0), stop=(i == DC // 2 - 1),
                        perf_mode=DR,
                    )
                rope_evict(pm, kT[fc][:, sl], cosk_t, sink_t, sl,
                           V_BK, V_BKR, fc)

        def v_unit(kc):
            for f2 in range(2):
                pm = ps1.tile([P, 512], F32, tag="mm", bufs=2,
                              name=f"pmv_{kc}_{f2}")
                for i in range(DC // 2):
                    nc.tensor.matmul(
                        pm[:], chatT[i][:, :, kc * P : (kc + 1) * P],
                        wv_sb[:, 2 * i : 2 * i + 2,
                              f2 * 512 : (f2 + 1) * 512],
                        start=(i == 0), stop=(i == DC // 2 - 1),
                        perf_mode=DR,
                    )
                nc.vector.tensor_add(
                    vext[kc][:, f2 * 8 : (f2 + 1) * 8, 0:HD],
                    pm[:].rearrange("p (h d) -> p h d", d=HD),
                    bvb[:, f2 * 512 : (f2 + 1) * 512].rearrange(
                        "p (h d) -> p h d", d=HD),
                )

        def q_unit(fc):
            for tt in range(TQ // 512):
                sl = slice(tt * 512, (tt + 1) * 512)
                pm = ps1.tile([P, 512], F32, tag="mm", bufs=2,
                              name=f"pmq_{fc}_{tt}")
                for i in range(DC // 2):
                    nc.tensor.matmul(
                        pm[:],
                        wq_sb[:, 2 * i : 2 * i + 2, fc * P : (fc + 1) * P],
                        xhatT[i][:, :, sl],
                        start=(i == 0), stop=(i == DC // 2 - 1),
                        perf_mode=DR,
                    )
                rope_evict(pm, qT[fc][:, sl], cosq_t, sinq_t, sl,
                           V_BQ, V_BQR, fc)

        # interleave: PE-heavy k/v units hide LN(x)/rope-k DVE+ActE work
        for fc in range(DC):
            k_unit(fc)
            v_unit(2 * fc)
            v_unit(2 * fc + 1)
            if fc < TQ // 512:
                ln_chunk(ps1, xin_load, xhatT, fc, "xhat", True, True)
        xin_cm.__exit__(None, None, None)
        ropek_cm.__exit__(None, None, None)
        chat_cm.__exit__(None, None, None)

        # q last (xin/ropek space freed); attention follows per-fc
        wqp_cm, wqp_p = _pool(tc, "wqp", 1, side="right")
        wq_sb = wqp_p.tile([P, DC, D], FP8, tag="wq8", name="wq8")
        nc.sync.dma_start(wq_sb[:], wq)
        ropeq_cm, ropeq_p = _pool(tc, "ropeq", 1, side="right")
        cosq_t = ropeq_p.tile([P, TQ], BF16, tag="cosq")
        nc.sync.dma_start(cosq_t[:], cosq)
        sinq_t = ropeq_p.tile([P, TQ], BF16, tag="sinq")
        nc.sync.dma_start(sinq_t[:], sinq)
        qT = [qT_p.tile([P, TQ], BF16, tag="qT", name=f"qT{i}")
              for i in range(DC)]
        for fc in range(DC):
            q_unit(fc)
        ropeq_cm.__exit__(None, None, None)
        wqp_cm.__exit__(None, None, None)
        xhat_cm.__exit__(None, None, None)
        w_cm.__exit__(None, None, None)
        ps1_cm.__exit__(None, None, None)

        # ================= phase 5: attention =================
        # Per (tt, head-pair): 16 kc steps. Each kc: two concurrent K=64
        # score matmuls (PE row tiles 0/64) into a 2-bank psum pair-tile,
        # one [128,1024] exp, two U accumulations (M=65, ones-column
        # denominator). Head pairs processed two at a time so softmax
        # reciprocals batch 4 heads per DVE call.
        ps5_cm, ps5 = _psum(tc, "ps5")
        # wo prefetched here so its DMA hides under attention
        wo_cm, wo_p = _pool(tc, "wo", DC, side="right")
        wo_sb = []
        for dc in range(DC):
            wt = wo_p.tile([P, D], BF16, tag="wo", name=f"wo{dc}")
            nc.sync.dma_start(wt[:], wo[dc * P : (dc + 1) * P, :])
            wo_sb.append(wt)
        attnT_cm, attnT_p = _pool(tc, "attnT", DC, side="right")
        e_cm, e_p = _pool(tc, "epool", 4)
        attnT = [attnT_p.tile([P, TQ], BF16, tag="attnT", name=f"attnT{i}")
                 for i in range(DC)]
        for tt in range(TQ // 512):
            sl = slice(tt * 512, (tt + 1) * 512)
            for hp2 in range(4):       # pairs of head-pairs
                pus = []
                for i in range(2):     # head pair index within group
                    fc = hp2 * 2 + i
                    pu = ps5.tile([P, 2, 512], F32, tag="u", bufs=2,
                                  name=f"pu_{tt}_{fc}")
                    pus.append(pu)
                    for kc in range(KC):
                        psc = ps5.tile([P, 2, 512], F32, tag="sc", bufs=2,
                                       name=f"psc_{tt}_{fc}_{kc}")
                        for j in range(2):   # head row-halves, concurrent
                            hb = j * HD
                            nc.tensor.matmul(
                                psc[:, j, :],
                                kT[fc][hb : hb + HD, kc * P : (kc + 1) * P],
                                qT[fc][hb : hb + HD, sl],
                                start=True, stop=True,
                            )
                        e = e_p.tile([P, 2, 512], BF16, tag="e",
                                     name=f"e_{tt}_{fc}_{kc}")
                        nc.scalar.activation(e[:], psc[:], AF.Exp, scale=0.125)
                        for j in range(2):
                            nc.tensor.matmul(
                                pu[0 : HD + 1, j, :],
                                vext[kc][:, fc * 2 + j, :],
                                e[:, j, :],
                                start=(kc == 0), stop=(kc == KC - 1),
                            )
                # softmax epilogue: copy den row to a partition-0 SBUF tile
                # (the custom-DVE reciprocal only honors base_partition 0),
                # then one fast-approx reciprocal for both heads
                for i in range(2):
                    fc = hp2 * 2 + i
                    den0 = stat.tile([1, 2, 512], F32, tag="den0", bufs=1,
                                     name=f"den0_{tt}_{fc}")
                    nc.vector.tensor_copy(
                        den0[:], pus[i][HD : HD + 1, :, :])
                    rcp = stat.tile([1, 2, 512], F32, tag="rcp", bufs=2,
                                    name=f"rcp_{tt}_{fc}")
                    nc.vector.reciprocal_approx_fast(rcp[:], den0[:])
                    for j in range(2):
                        hb = j * HD
                        rb = work.tile([HD, 512], F32, tag="rb", bufs=2,
                                       name=f"rb_{tt}_{fc}_{j}")
                        nc.gpsimd.partition_broadcast(rb[:], rcp[:, j, :])
                        nc.vector.tensor_mul(
                            attnT[fc][hb : hb + HD, sl],
                            pus[i][0:HD, j, :], rb[:])
        if DBG:
            dbg_dump(attnT[0][:], d_attnT, TQ)
        e_cm.__exit__(None, None, None)
        vext_cm.__exit__(None, None, None)
        kT_cm.__exit__(None, None, None)
        qT_cm.__exit__(None, None, None)
        ps5_cm.__exit__(None, None, None)

        # ================= phase 6: x2^T = Wo^T attn + x^T + bo ========
        ps6_cm, ps6 = _psum(tc, "ps6")
        x2_cm, x2_p = _pool(tc, "x2", DC)
        xin6_cm, xin6_p = _pool(tc, "xin6", 4, side="right")
        xhat2_cm, xhat2_p = _pool(tc, "xhat2", DC, side="right")
        h1_cm, h1_p = _pool(tc, "h1", 2 * NHC)
        w1_cm, w1_p = _pool(tc, "w1s", 4)
        x2T = [x2_p.tile([P, TQ], BF16, tag="x2", name=f"x2T{i}")
               for i in range(DC)]
        xhat2T = ln_alloc(xhat2_p, "xhat2", TQ, False)
        h1 = {}

        def outproj_unit(tt, fc):
            sl = slice(tt * 512, (tt + 1) * 512)
            xres = xin6_p.tile([P, 512], F32R, tag="xin6",
                               name=f"xres_{fc}_{tt}")
            nc.sync.dma_start(xres[:], xT[fc * P : (fc + 1) * P, sl])
            pm = ps6.tile([P, 512], F32, tag="mm", bufs=2,
                          name=f"pmo_{fc}_{tt}")
            for dc in range(DC):
                nc.tensor.matmul(
                    pm[:], wo_sb[dc][:, fc * P : (fc + 1) * P],
                    attnT[dc][:, sl], start=(dc == 0), stop=(dc == DC - 1),
                )
            nc.vector.scalar_tensor_tensor(
                x2T[fc][:, sl], pm[:], gcol(V_BO, fc),
                xres[:].bitcast(F32),
                ALU.add, ALU.add,
            )

        def w1_unit(tt, hc):
            sl = slice(tt * 512, (tt + 1) * 512)
            w1t = w1_p.tile([P, DC, P], BF16, tag="w1",
                            name=f"w1_{tt}_{hc}")
            nc.sync.dma_start(w1t[:], w1[:, hc, :, :])
            ph = ps6.tile([P, 512], F32, tag="mm", bufs=2,
                          name=f"ph1_{tt}_{hc}")
            for dc in range(DC):
                nc.tensor.matmul(
                    ph[:], w1t[:, dc, :],
                    xhat2T[dc][:, sl],
                    start=(dc == 0), stop=(dc == DC - 1),
                )
            ht = h1_p.tile([P, 512], BF16, tag="h1", name=f"h1_{tt}_{hc}")
            nc.scalar.activation(ht[:], ph[:], AF.Gelu,
                                 bias=b1t[:, hc : hc + 1])
            h1[tt, hc] = ht

        x2_load = lambda dc, tt: x2T[dc][:, tt * 512 : (tt + 1) * 512]
        for fc in range(DC):
            outproj_unit(0, fc)
        ln_chunk(ps6, x2_load, xhat2T, 0, "xhat2", False, False)
        # W1(tt0) PE work hides outproj(tt1)/LN2(tt1) DVE+ActE work
        for fc in range(DC):
            outproj_unit(1, fc)
            for hc in range(4 * fc, 4 * fc + 4):
                w1_unit(0, hc)
        ln_chunk(ps6, x2_load, xhat2T, 1, "xhat2", False, False)
        for hc in range(NHC):
            w1_unit(1, hc)
        if DBG:
            dbg_dump(x2T[0][:], d_x2T, TQ)
        w1_cm.__exit__(None, None, None)
        xhat2_cm.__exit__(None, None, None)
        xin6_cm.__exit__(None, None, None)
        attnT_cm.__exit__(None, None, None)
        wo_cm.__exit__(None, None, None)
        ps6_cm.__exit__(None, None, None)

        # ================= phase 8b: MLP down-proj, 8-bank pass ========
        ps8_cm, ps8 = _psum(tc, "ps8")
        w2_cm, w2_p = _pool(tc, "w2s", 5)
        out_cm, out_p = _pool(tc, "ostage", 4)
        for tt in range(TQ // 512):
            sl = slice(tt * 512, (tt + 1) * 512)
            pms = ps8.tile([P, DC, 512], F32, tag="mlp8", bufs=1,
                           name=f"pmh2_{tt}")
            for hc in range(NHC):
                w2t = w2_p.tile([P, D], BF16, tag="w2",
                                name=f"w2_{tt}_{hc}")
                nc.sync.dma_start(w2t[:], w2[hc * P : (hc + 1) * P, :])
                for fc in range(DC):
                    nc.tensor.matmul(
                        pms[:, fc, :], w2t[:, fc * P : (fc + 1) * P],
                        h1[tt, hc][:], start=(hc == 0), stop=(hc == NHC - 1),
                    )
            for fc in range(DC):
                ot = out_p.tile([P, 512], F32, tag="ostage",
                                name=f"ot_{tt}_{fc}")
                nc.vector.scalar_tensor_tensor(
                    ot[:], pms[:, fc, :], gcol(V_B2, fc), x2T[fc][:, sl],
                    ALU.add, ALU.add,
                )
                nc.sync.dma_start(outT[fc * P : (fc + 1) * P, sl], ot[:])

        out_cm.__exit__(None, None, None)
        w2_cm.__exit__(None, None, None)
        h1_cm.__exit__(None, None, None)
        x2_cm.__exit__(None, None, None)
        ps8_cm.__exit__(None, None, None)
        stat_cm.__exit__(None, None, None)
        work_cm.__exit__(None, None, None)
        const_cm.__exit__(None, None, None)

    nc.compile()
    return nc


# old feature index (within a 64-dim head block) at each new position:
# pairs (j, j+32) become adjacent (2j, 2j+1)
OLD_OF_NEW = np.array([j // 2 if j % 2 == 0 else j // 2 + 32
                       for j in range(HD)])


def _perm_cols(a):
    """Permute the last dim (64-multiple) per 64-feature head block."""
    a = np.asarray(a, np.float32)
    shp = a.shape
    nb = shp[-1] // HD
    a = a.reshape(shp[:-1] + (nb, HD))
    a = a[..., OLD_OF_NEW]
    return a.reshape(shp)


def _pairswap(a):
    """Swap even/odd positions of the last dim."""
    a = np.asarray(a, np.float32)
    shp = a.shape
    a = a.reshape(shp[:-1] + (shp[-1] // 2, 2))
    a = a[..., ::-1]
    return np.ascontiguousarray(a.reshape(shp))


def _col8(v):
    return np.ascontiguousarray(
        np.asarray(v, np.float32).reshape(DC, P).T.astype(np.float32))


def make_in_maps(inputs):
    x = np.asarray(inputs["x"], np.float32)
    context = np.asarray(inputs["context"], np.float32)
    cos = np.asarray(inputs["rope_cos"], np.float32).reshape(L, HD)
    sin = np.asarray(inputs["rope_sin"], np.float32).reshape(L, HD)

    bf = lambda a: np.ascontiguousarray(np.asarray(a, np.float32)).astype(
        ml_dtypes.bfloat16)
    Wq = np.asarray(inputs["Wq"], np.float32)
    Wkv = np.asarray(inputs["Wkv"], np.float32)
    W1 = np.asarray(inputs["W1"], np.float32)
    g_q = np.asarray(inputs["g_q"], np.float32)
    be_q = np.asarray(inputs["be_q"], np.float32)
    g_kv = np.asarray(inputs["g_kv"], np.float32)
    be_kv = np.asarray(inputs["be_kv"], np.float32)
    g_ffn = np.asarray(inputs["g_ffn"], np.float32)
    be_ffn = np.asarray(inputs["be_ffn"], np.float32)

    # fold LN gamma/beta into weights/biases
    Wq_f = g_q[:, None] * Wq
    bq_f = be_q @ Wq + np.asarray(inputs["bq"], np.float32)
    Wk_f = g_kv[:, None] * Wkv[:, :D]
    bk_f = be_kv @ Wkv[:, :D] + np.asarray(inputs["bkv"], np.float32)[:D]
    Wv_f = g_kv[:, None] * Wkv[:, D:]
    bv_f = be_kv @ Wkv[:, D:] + np.asarray(inputs["bkv"], np.float32)[D:]
    W1_f = g_ffn[:, None] * W1
    b1_f = be_ffn @ W1 + np.asarray(inputs["b1"], np.float32)

    # rope pair permutation on q/k output features
    Wq_p = _perm_cols(Wq_f)
    bq_p = _perm_cols(bq_f)
    Wk_p = _perm_cols(Wk_f)
    bk_p = _perm_cols(bk_f)

    # q/k/v weights: fp8e4m3 packed [p, dc, m] = W[dc*128+p, m]
    f8 = lambda a: np.ascontiguousarray(
        np.asarray(a, np.float32).reshape(DC, P, D).transpose(1, 0, 2)
    ).astype(ml_dtypes.float8_e4m3fn)
    wq_b = f8(Wq_p)
    wk_b = f8(Wk_p)
    wv_b = f8(Wv_f)
    wo_b = bf(inputs["Wo"])
    # w1 packed [p, hc, dc, j] = W1[dc*128+p, hc*128+j]
    w1_b = bf(np.ascontiguousarray(
        W1_f.reshape(DC, P, NHC, P).transpose(1, 2, 0, 3)))
    w2_b = bf(inputs["W2"])

    vecs = np.stack(
        [_col8(bq_p), _col8(bk_p),
         _col8(inputs["bo"]), _col8(inputs["b2"]),
         _col8(_pairswap(bq_p)), _col8(_pairswap(bk_p))],
        axis=1,
    )  # [128, 6, 8]
    vecs = np.ascontiguousarray(vecs)
    b1t = np.ascontiguousarray(b1_f.reshape(NHC, P).T)
    bvrow = np.ascontiguousarray(bv_f.reshape(1, D))

    # rope tables in permuted feature space:
    # cosP[n] = cos[old_of_new[n]]; sinE[2j] = -sin[j], sinE[2j+1] = sin[j+32]
    cosP = cos[:, OLD_OF_NEW]                        # [L, 64]
    sinP = sin[:, OLD_OF_NEW]
    sinE = sinP.copy()
    sinE[:, 0::2] = -sinE[:, 0::2]
    cosT = cosP.T                                    # [64, L]
    sinT = sinE.T
    cosk_full = np.ascontiguousarray(np.concatenate([cosT, cosT], 0)).astype(
        ml_dtypes.bfloat16)
    sink_full = np.ascontiguousarray(np.concatenate([sinT, sinT], 0)).astype(
        ml_dtypes.bfloat16)

    in_maps = []
    for c in range(NCORES):
        b, hf = c // 2, c % 2
        tsl = slice(hf * TQ, (hf + 1) * TQ)
        in_maps.append({
            "xT": np.ascontiguousarray(x[b, tsl, :].T),
            "ctxT": np.ascontiguousarray(context[b].T),
            "cosq": np.ascontiguousarray(cosk_full[:, tsl]),
            "sinq": np.ascontiguousarray(sink_full[:, tsl]),
            "cosk": cosk_full,
            "sink": sink_full,
            "wq": wq_b, "wk": wk_b, "wv": wv_b, "wo": wo_b,
            "w1": w1_b, "w2": w2_b,
            "vecs": vecs, "b1t": b1t, "bvrow": bvrow,
            "onesr": np.ones((P, 1), np.float32),
        })
    return in_maps


def kernel(**inputs) -> np.ndarray:
    global _CACHED_NC
    if _CACHED_NC is None:
        _CACHED_NC = build_nc()
    nc = _CACHED_NC
    in_maps = make_in_maps(inputs)
    res = run_bass_kernel_spmd(nc, in_maps, core_ids=list(range(NCORES)))
    out = np.empty((B, L, D), np.float32)
    for c in range(NCORES):
        b, hf = c // 2, c % 2
        out[b, hf * TQ : (hf + 1) * TQ, :] = res.results[c]["outT"].T
    return out



# revision 68
# speedup vs baseline: 1.0330x; 1.0096x over previous
"""Cross-attention transformer block on 8 TRN2 NeuronCores.

Sharding: 8 cores = 4 batches x 2 sequence-halves. Core c handles batch
b = c//2, query tokens [hf*1024, (hf+1)*1024) with hf = c%2. Each core
computes the FULL kv projection for its batch (duplicated across the 2
cores of a batch) so no collectives are needed.

Feature-major layout ([feature, token]) so matmuls contract over the
partition dim with natural weight layouts. Optimizations on top of the
v2 baseline (915us -> ~845us on the fast clock state):
  - q/k/v projections run fp8e4m3 with DoubleRow (256-deep contraction
    per pass, ~1.8x PE); LN outputs are emitted as [128, 2, T] fp8
    pair-tiles so DoubleRow slices them directly. v (vext) stays fp8
    as the U-matmul stationary. W1/W2/Wo remain bf16: fp8 there pushed
    rel err to ~2e-2 (the MLP path has no averaging to wash out
    quantization noise; attention does).
  - LN normalize and RoPE run on bf16 DVE 2x mode; ActE evicts psum
    to bf16 first (ActE is idle in those phases). Rope tables bf16.
  - All reciprocals use the custom-DVE reciprocal_approx_fast (~5x
    over InstReciprocal, which cost 3.3us per [1,512] row). NOTE: the
    custom op only honors base_partition 0 - softmax denominators are
    first copied from psum partition 64 to a partition-0 SBUF row.
  - Emission interleaves LN(x) chunks into the k/v unit stream so the
    LN DVE/ActE work hides under projection PE work; q is emitted last
    so attention (which needs qT[fc] ascending) can start early.
  - Attention: scores as two concurrent K=64 row-tiled matmuls; exp
    [128,1024] per head-pair on ActE (attention is ActE-bound: 284us
    of exp at 1 elem/lane/cycle is the phase floor); U with an
    appended ones-column (M=65) accumulates both U and the softmax
    denominator.
  - Weight DMA prefetch is emitted behind the first input chunk's
    loads; W1/W2 stream with 4-5 deep pools.
"""

import numpy as np
import ml_dtypes

import concourse.bass as bass
import concourse.bacc as bacc
import concourse.mybir as mybir
import concourse.tile as tile
from concourse.bass_utils import run_bass_kernel_spmd

F32 = mybir.dt.float32
F32R = mybir.dt.float32r
BF16 = mybir.dt.bfloat16
FP8 = mybir.dt.float8e4
AF = mybir.ActivationFunctionType
ALU = mybir.AluOpType
DR = mybir.MatmulPerfMode.DoubleRow

B, L, D, H, HD = 4, 2048, 1024, 16, 64
TQ = 1024          # query tokens per core
TK = 2048          # kv tokens per core
HID = 4 * D
NCORES = 8
P = 128
DC = D // P        # 8 feature chunks
KC = TK // P       # 16 kv-token chunks
NHC = HID // P     # 32 hidden chunks
EPS = 1e-5

# vecs[:, i, :] packing indices
(V_BQ, V_BK, V_BO, V_B2, V_BQR, V_BKR) = range(6)

PAIRSWAP_MASK = [i + 1 if i % 2 == 0 else i - 1 for i in range(32)]

_CACHED_NC = None


def _pool(tc, name, bufs, side="left"):
    cm = tc.tile_pool(name=name, bufs=bufs, side=side)
    return cm, cm.__enter__()


def _psum(tc, name):
    cm = tc.tile_pool(name=name, bufs=1, space="PSUM")
    return cm, cm.__enter__()


def build_nc():
    nc = bacc.Bacc("TRN2", debug=False, num_devices=NCORES)

    xT = nc.declare_dram_parameter("xT", [D, TQ], F32R, False).ap()
    ctxT = nc.declare_dram_parameter("ctxT", [D, TK], F32R, False).ap()
    cosq = nc.declare_dram_parameter("cosq", [P, TQ], BF16, False).ap()
    sinq = nc.declare_dram_parameter("sinq", [P, TQ], BF16, False).ap()
    cosk = nc.declare_dram_parameter("cosk", [P, TK], BF16, False).ap()
    sink = nc.declare_dram_parameter("sink", [P, TK], BF16, False).ap()
    # q/k/v weights packed [p, dc, m] = W[dc*128+p, m], fp8 for DoubleRow
    wq = nc.declare_dram_parameter("wq", [P, DC, D], FP8, False).ap()
    wk = nc.declare_dram_parameter("wk", [P, DC, D], FP8, False).ap()
    wv = nc.declare_dram_parameter("wv", [P, DC, D], FP8, False).ap()
    wo = nc.declare_dram_parameter("wo", [D, D], BF16, False).ap()
    # w1 packed [p, hc, dc, j] = W1[dc*128+p, hc*128+j]
    w1 = nc.declare_dram_parameter("w1", [P, NHC, DC, P], BF16, False).ap()
    w2 = nc.declare_dram_parameter("w2", [HID, D], BF16, False).ap()
    vecs_d = nc.declare_dram_parameter("vecs", [P, 6, DC], F32, False).ap()
    b1t_d = nc.declare_dram_parameter("b1t", [P, NHC], F32, False).ap()
    bvrow_d = nc.declare_dram_parameter("bvrow", [1, D], F32, False).ap()
    onesr_d = nc.declare_dram_parameter("onesr", [P, 1], F32R, False).ap()
    outT = nc.declare_dram_parameter("outT", [D, TQ], F32, True).ap()
    import os
    DBG = os.environ.get("KDBG", "0") == "1"
    if DBG:
        d_chat = nc.declare_dram_parameter("d_chat", [P, TK], F32, True).ap()
        d_xhat = nc.declare_dram_parameter("d_xhat", [P, TQ], F32, True).ap()
        d_qT = nc.declare_dram_parameter("d_qT", [P, TQ], F32, True).ap()
        d_kT = nc.declare_dram_parameter("d_kT", [P, TK], F32, True).ap()
        d_attnT = nc.declare_dram_parameter("d_attnT", [P, TQ], F32, True).ap()
        d_x2T = nc.declare_dram_parameter("d_x2T", [P, TQ], F32, True).ap()
        d_pu = nc.declare_dram_parameter("d_pu", [P, 512], F32, True).ap()
        d_rcp = nc.declare_dram_parameter("d_rcp", [1, 512], F32, True).ap()
        d_rb = nc.declare_dram_parameter("d_rb", [1, 512], F32, True).ap()

    with tile.TileContext(nc) as tc:
        const_cm, const = _pool(tc, "const", 1)
        work_cm, work = _pool(tc, "work", 8)       # f32 [128,512] scratch
        stat_cm, stat = _pool(tc, "stat", 4)

        # ---- constants ----
        vecs = const.tile([P, 6, DC], F32, tag="vecs")
        nc.sync.dma_start(vecs[:], vecs_d)
        b1t = const.tile([P, NHC], F32, tag="b1t")
        nc.sync.dma_start(b1t[:], b1t_d)
        bvrow = const.tile([1, D], F32, tag="bvrow")
        nc.sync.dma_start(bvrow[:], bvrow_d)
        bvb = const.tile([P, D], F32, tag="bvb")
        nc.gpsimd.partition_broadcast(bvb[:], bvrow[:])
        onesP = const.tile([P, 1], F32, tag="onesP")
        nc.vector.memset(onesP[:], 1.0)
        onesPr = const.tile([P, 1], F32R, tag="onesPr")
        nc.sync.dma_start(onesPr[:], onesr_d)
        onesPb = const.tile([P, 1], BF16, tag="onesPb")
        nc.vector.memset(onesPb[:], 1.0)
        eps1 = const.tile([1, 1], F32, tag="eps1")
        nc.vector.memset(eps1[:], EPS)

        def scratch(name):
            return work.tile([P, 512], F32, tag="scratch", name=name)

        def gcol(idx, dc):
            return vecs[:, idx, dc : dc + 1]

        def ln_alloc(out_pool, out_tag, nt, paired):
            if paired:
                return [out_pool.tile([P, 2, nt], FP8, tag=out_tag,
                                      name=f"{out_tag}{i}")
                        for i in range(DC // 2)]
            return [out_pool.tile([P, nt], BF16, tag=out_tag,
                                  name=f"{out_tag}{i}") for i in range(DC)]

        def ln_chunk(ps, load_fn, outs, tt, out_tag, src_r, paired):
            """One 512-token LayerNorm chunk (gamma/beta folded host-side)."""
            if True:
                sl = slice(tt * 512, (tt + 1) * 512)
                raw = [load_fn(dc, tt) for dc in range(DC)]
                srcs = [r.bitcast(F32) if src_r else r for r in raw]
                pr_row = ps.tile([P, 512], F32, tag="row", bufs=1,
                                 name=f"lnrow_{out_tag}_{tt}")
                # sum on partition 0, sumsq on partition 32 (same bank)
                for dc in range(DC):
                    sq = work.tile([P, 512], BF16, tag="sq", bufs=2,
                                   name=f"sq_{out_tag}_{tt}_{dc}")
                    nc.scalar.square(sq[:], srcs[dc])
                    nc.tensor.matmul(
                        pr_row[0:1, :],
                        onesPr[:] if src_r else onesPb[:],
                        raw[dc],
                        start=(dc == 0), stop=(dc == DC - 1),
                    )
                    nc.tensor.matmul(
                        pr_row[32:33, :], onesPb[:],
                        sq[:],
                        start=(dc == 0), stop=(dc == DC - 1),
                    )
                st = stat.tile([1, 3, 512], F32, tag="stats", bufs=2,
                               name=f"st_{out_tag}_{tt}")
                mu, var, rs = (st[:, i, :] for i in range(3))
                nc.vector.tensor_scalar_mul(mu, pr_row[0:1, :], 1.0 / D)
                nc.vector.tensor_scalar_mul(rs, pr_row[32:33, :], 1.0 / D)
                nc.vector.tensor_mul(var, mu, mu)
                nc.vector.tensor_sub(var, rs, var)
                # rs <- sqrt(var+eps) then var <- 1/rs (fast approx)
                nc.scalar.activation(rs, var, AF.Sqrt, bias=eps1[:])
                nc.vector.reciprocal_approx_fast(var, rs)
                rs = var
                # bf16 stats rows -> bf16 broadcasts -> bf16 2x normalize
                stb = stat.tile([1, 2, 512], BF16, tag="statsb", bufs=1,
                                name=f"stb_{out_tag}_{tt}")
                mu_b, rs_b = stb[:, 0, :], stb[:, 1, :]
                nc.vector.tensor_copy(mu_b, mu)
                nc.vector.tensor_copy(rs_b, rs)
                mub = work.tile([P, 512], BF16, tag="mub", bufs=2,
                                name=f"mub_{out_tag}_{tt}")
                nc.gpsimd.partition_broadcast(mub[:], mu_b)
                rsb = work.tile([P, 512], BF16, tag="rsb", bufs=2,
                                name=f"rsb_{out_tag}_{tt}")
                nc.gpsimd.partition_broadcast(rsb[:], rs_b)
                for dc in range(DC):
                    # ActE evicts src to bf16 so both DVE ops run 2x mode
                    xb = work.tile([P, 512], BF16, tag="xb", bufs=2,
                                   name=f"xb_{out_tag}_{tt}_{dc}")
                    nc.scalar.activation(xb[:], srcs[dc], AF.Copy)
                    t = work.tile([P, 512], BF16, tag="lnt", bufs=2,
                                  name=f"lnt_{out_tag}_{tt}_{dc}")
                    nc.vector.tensor_sub(t[:], xb[:], mub[:])
                    dst = (outs[dc // 2][:, dc % 2, sl] if paired
                           else outs[dc][:, sl])
                    nc.vector.tensor_mul(dst, t[:], rsb[:])

        def ln_T(ps, load_fn, nt, out_pool, out_tag, src_r, paired=False):
            outs = ln_alloc(out_pool, out_tag, nt, paired)
            for tt in range(nt // 512):
                ln_chunk(ps, load_fn, outs, tt, out_tag, src_r, paired)
            return outs

        def rope_evict(psum, out_ap, cos_t, sin_t, sl, b_idx, b_rot_idx, fc):
            """out = (psum + b)*cosP + pairswap(psum + b)*sinE (bf16).

            bR = pairswap(b), so applying b on the ActE eviction (bias is
            per-partition there, free) makes the shuffled copy carry bR
            automatically; both DVE multiplies then run as bf16 2x TT
            (STT has no 2x uop and costs ~745ns vs 345ns for TT).
            """
            pb = work.tile([P, 512], BF16, tag="ropePb", bufs=2,
                           name=f"ropeP_{b_idx}_{fc}_{sl.start}")
            nc.scalar.activation(pb[:], psum[:], AF.Identity,
                                 bias=gcol(b_idx, fc))
            sh = work.tile([P, 512], BF16, tag="ropeSh", bufs=2,
                           name=f"ropeS_{b_idx}_{fc}_{sl.start}")
            nc.vector.stream_shuffle(sh[:], pb[:], PAIRSWAP_MASK)
            t = work.tile([P, 512], BF16, tag="ropeT", bufs=2,
                          name=f"ropeA_{b_idx}_{fc}_{sl.start}")
            nc.vector.tensor_mul(t[:], pb[:], cos_t[:, sl])
            t2 = work.tile([P, 512], BF16, tag="ropeT", bufs=2,
                           name=f"ropeB_{b_idx}_{fc}_{sl.start}")
            nc.vector.tensor_mul(t2[:], sh[:], sin_t[:, sl])
            nc.vector.tensor_add(out_ap, t[:], t2[:])

        def dram_loader(pool, dram_ap, tag):
            def load(dc, tt):
                t = pool.tile([P, 512], F32R, tag=tag,
                              name=f"{tag}_{dc}_{tt}")
                nc.sync.dma_start(
                    t[:], dram_ap[dc * P : (dc + 1) * P,
                                  tt * 512 : (tt + 1) * 512])
                return t[:]
            return load

        ps1_cm, ps1 = _psum(tc, "ps1")

        def dbg_dump(src_ap, dram_ap, n):
            if not DBG:
                return
            for c in range(n // 512):
                s = work.tile([P, 512], F32, tag="dbgs", bufs=2,
                              name=f"dbg_{dram_ap}_{c}")
                nc.vector.tensor_copy(
                    s[:], src_ap[:, c * 512 : (c + 1) * 512])
                nc.sync.dma_start(
                    dram_ap[:, c * 512 : (c + 1) * 512], s[:])

        # ====== front: LN(ctx), then {LN(x) | k | v | q} interleaved ======
        # weights prefetched up front so DMA hides under LN compute
        w_cm, w_p = _pool(tc, "wqkv", 8, side="right")
        chat_cm, chat_p = _pool(tc, "chat", DC, side="right")
        # left-stack order chosen for LIFO exits:
        # qT/kT/vext (die after attention) below xhat (dies after q),
        # then cin (dies after ctx-LN), then xin (dies after interleave)
        qT_cm, qT_p = _pool(tc, "qT", DC)
        kT_cm, kT_p = _pool(tc, "kT", DC)
        vext_cm, vext_p = _pool(tc, "vext", KC)
        xhat_cm, xhat_p = _pool(tc, "xhat", DC)

        cin_cm, cin_p = _pool(tc, "cin", 12)
        chatT = ln_alloc(chat_p, "chat", TK, paired=True)
        cin_load = dram_loader(cin_p, ctxT, "cin")
        wk_sb = wv_sb = None
        for tt in range(TK // 512):
            ln_chunk(ps1, cin_load, chatT, tt, "chat", True, True)
            if tt == 0:
                # weight prefetch behind the first input chunk's DMAs
                wk_sb = w_p.tile([P, DC, D], FP8, tag="w8", bufs=2,
                                 name="wk8")
                nc.sync.dma_start(wk_sb[:], wk)
                wv_sb = w_p.tile([P, DC, D], FP8, tag="w8", bufs=2,
                                 name="wv8")
                nc.sync.dma_start(wv_sb[:], wv)
        cin_cm.__exit__(None, None, None)

        ropek_cm, ropek_p = _pool(tc, "ropek", 1, side="right")
        cosk_t = ropek_p.tile([P, TK], BF16, tag="cosk")
        nc.sync.dma_start(cosk_t[:], cosk)
        sink_t = ropek_p.tile([P, TK], BF16, tag="sink")
        nc.sync.dma_start(sink_t[:], sink)

        xin_cm, xin_p = _pool(tc, "xin", 8)
        xhatT = ln_alloc(xhat_p, "xhat", TQ, paired=True)
        xin_load = dram_loader(xin_p, xT, "xin")
        kT = [kT_p.tile([P, TK], BF16, tag="kT", name=f"kT{i}")
              for i in range(DC)]
        vext = []
        for kc in range(KC):
            vt = vext_p.tile([P, H, HD + 1], FP8, tag="vext",
                             name=f"vext{kc}")
            nc.vector.memset(vt[:, :, HD : HD + 1], 1.0)
            vext.append(vt)

        def k_unit(fc):
            for tt in range(TK // 512):
                sl = slice(tt * 512, (tt + 1) * 512)
                pm = ps1.tile([P, 512], F32, tag="mm", bufs=2,
                              name=f"pmk_{fc}_{tt}")
                for i in range(DC // 2):
                    nc.tensor.matmul(
                        pm[:],
                        wk_sb[:, 2 * i : 2 * i + 2, fc * P : (fc + 1) * P],
                        chatT[i][:, :, sl],
                        start=(i == 0), stop=(i == DC // 2 - 1),
                        perf_mode=DR,
                    )
                rope_evict(pm, kT[fc][:, sl], cosk_t, sink_t, sl,
                           V_BK, V_BKR, fc)

        def v_unit(kc):
            for f2 in range(2):
                pm = ps1.tile([P, 512], F32, tag="mm", bufs=2,
                              name=f"pmv_{kc}_{f2}")
                for i in range(DC // 2):
                    nc.tensor.matmul(
                        pm[:], chatT[i][:, :, kc * P : (kc + 1) * P],
                        wv_sb[:, 2 * i : 2 * i + 2,
                              f2 * 512 : (f2 + 1) * 512],
                        start=(i == 0), stop=(i == DC // 2 - 1),
                        perf_mode=DR,
                    )
                nc.vector.tensor_add(
                    vext[kc][:, f2 * 8 : (f2 + 1) * 8, 0:HD],
                    pm[:].rearrange("p (h d) -> p h d", d=HD),
                    bvb[:, f2 * 512 : (f2 + 1) * 512].rearrange(
                        "p (h d) -> p h d", d=HD),
                )

        def q_unit(fc):
            for tt in range(TQ // 512):
                sl = slice(tt * 512, (tt + 1) * 512)
                pm = ps1.tile([P, 512], F32, tag="mm", bufs=2,
                              name=f"pmq_{fc}_{tt}")
                for i in range(DC // 2):
                    nc.tensor.matmul(
                        pm[:],
                        wq_sb[:, 2 * i : 2 * i + 2, fc * P : (fc + 1) * P],
                        xhatT[i][:, :, sl],
                        start=(i == 0), stop=(i == DC // 2 - 1),
                        perf_mode=DR,
                    )
                rope_evict(pm, qT[fc][:, sl], cosq_t, sinq_t, sl,
                           V_BQ, V_BQR, fc)

        # interleave: PE-heavy k/v units hide LN(x)/rope-k DVE+ActE work
        for fc in range(DC):
            k_unit(fc)
            v_unit(2 * fc)
            v_unit(2 * fc + 1)
            if fc < TQ // 512:
                ln_chunk(ps1, xin_load, xhatT, fc, "xhat", True, True)
        xin_cm.__exit__(None, None, None)
        ropek_cm.__exit__(None, None, None)
        chat_cm.__exit__(None, None, None)

        # q last (xin/ropek space freed)
        wqp_cm, wqp_p = _pool(tc, "wqp", 1, side="right")
        wq_sb = wqp_p.tile([P, DC, D], FP8, tag="wq8", name="wq8")
        nc.sync.dma_start(wq_sb[:], wq)
        ropeq_cm, ropeq_p = _pool(tc, "ropeq", 1, side="right")
        cosq_t = ropeq_p.tile([P, TQ], BF16, tag="cosq")
        nc.sync.dma_start(cosq_t[:], cosq)
        sinq_t = ropeq_p.tile([P, TQ], BF16, tag="sinq")
        nc.sync.dma_start(sinq_t[:], sinq)
        qT = [qT_p.tile([P, TQ], BF16, tag="qT", name=f"qT{i}")
              for i in range(DC)]
        for fc in range(DC):
            q_unit(fc)
        ropeq_cm.__exit__(None, None, None)
        wqp_cm.__exit__(None, None, None)
        xhat_cm.__exit__(None, None, None)
        w_cm.__exit__(None, None, None)
        ps1_cm.__exit__(None, None, None)

        # ================= phase 5: attention =================
        # Per (tt, head-pair): 16 kc steps. Each kc: two concurrent K=64
        # score matmuls (PE row tiles 0/64) into a 2-bank psum pair-tile,
        # one [128,1024] exp, two U accumulations (M=65, ones-column
        # denominator). Head pairs processed two at a time so softmax
        # reciprocals batch 4 heads per DVE call.
        ps5_cm, ps5 = _psum(tc, "ps5")
        # wo prefetched here so its DMA hides under attention
        wo_cm, wo_p = _pool(tc, "wo", DC, side="right")
        wo_sb = []
        for dc in range(DC):
            wt = wo_p.tile([P, D], BF16, tag="wo", name=f"wo{dc}")
            nc.sync.dma_start(wt[:], wo[dc * P : (dc + 1) * P, :])
            wo_sb.append(wt)
        attnT_cm, attnT_p = _pool(tc, "attnT", DC, side="right")
        e_cm, e_p = _pool(tc, "epool", 4)
        attnT = [attnT_p.tile([P, TQ], BF16, tag="attnT", name=f"attnT{i}")
                 for i in range(DC)]
        for tt in range(TQ // 512):
            sl = slice(tt * 512, (tt + 1) * 512)
            for hp2 in range(4):       # pairs of head-pairs
                pus = []
                for i in range(2):     # head pair index within group
                    fc = hp2 * 2 + i
                    pu = ps5.tile([P, 2, 512], F32, tag="u", bufs=2,
                                  name=f"pu_{tt}_{fc}")
                    pus.append(pu)
                    for kc in range(KC):
                        psc = ps5.tile([P, 2, 512], F32, tag="sc", bufs=2,
                                       name=f"psc_{tt}_{fc}_{kc}")
                        for j in range(2):   # head row-halves, concurrent
                            hb = j * HD
                            nc.tensor.matmul(
                                psc[:, j, :],
                                kT[fc][hb : hb + HD, kc * P : (kc + 1) * P],
                                qT[fc][hb : hb + HD, sl],
                                start=True, stop=True,
                            )
                        e = e_p.tile([P, 2, 512], BF16, tag="e",
                                     name=f"e_{tt}_{fc}_{kc}")
                        nc.scalar.activation(e[:], psc[:], AF.Exp, scale=0.125)
                        for j in range(2):
                            nc.tensor.matmul(
                                pu[0 : HD + 1, j, :],
                                vext[kc][:, fc * 2 + j, :],
                                e[:, j, :],
                                start=(kc == 0), stop=(kc == KC - 1),
                            )
                # softmax epilogue: copy den row to a partition-0 SBUF tile
                # (the custom-DVE reciprocal only honors base_partition 0),
                # then one fast-approx reciprocal for both heads
                for i in range(2):
                    fc = hp2 * 2 + i
                    den0 = stat.tile([1, 2, 512], F32, tag="den0", bufs=1,
                                     name=f"den0_{tt}_{fc}")
                    nc.vector.tensor_copy(
                        den0[:], pus[i][HD : HD + 1, :, :])
                    rcp = stat.tile([1, 2, 512], F32, tag="rcp", bufs=2,
                                    name=f"rcp_{tt}_{fc}")
                    nc.vector.reciprocal_approx_fast(rcp[:], den0[:])
                    for j in range(2):
                        hb = j * HD
                        rb = work.tile([HD, 512], F32, tag="rb", bufs=2,
                                       name=f"rb_{tt}_{fc}_{j}")
                        nc.gpsimd.partition_broadcast(rb[:], rcp[:, j, :])
                        nc.vector.tensor_mul(
                            attnT[fc][hb : hb + HD, sl],
                            pus[i][0:HD, j, :], rb[:])
        if DBG:
            dbg_dump(attnT[0][:], d_attnT, TQ)
        e_cm.__exit__(None, None, None)
        vext_cm.__exit__(None, None, None)
        kT_cm.__exit__(None, None, None)
        qT_cm.__exit__(None, None, None)
        ps5_cm.__exit__(None, None, None)

        # ================= phase 6: x2^T = Wo^T attn + x^T + bo ========
        ps6_cm, ps6 = _psum(tc, "ps6")
        x2_cm, x2_p = _pool(tc, "x2", DC)
        xin6_cm, xin6_p = _pool(tc, "xin6", 4, side="right")
        xhat2_cm, xhat2_p = _pool(tc, "xhat2", DC, side="right")
        h1_cm, h1_p = _pool(tc, "h1", 2 * NHC)
        w1_cm, w1_p = _pool(tc, "w1s", 4)
        x2T = [x2_p.tile([P, TQ], BF16, tag="x2", name=f"x2T{i}")
               for i in range(DC)]
        xhat2T = ln_alloc(xhat2_p, "xhat2", TQ, False)
        h1 = {}

        def outproj_unit(tt, fc):
            sl = slice(tt * 512, (tt + 1) * 512)
            xres = xin6_p.tile([P, 512], F32R, tag="xin6",
                               name=f"xres_{fc}_{tt}")
            nc.sync.dma_start(xres[:], xT[fc * P : (fc + 1) * P, sl])
            pm = ps6.tile([P, 512], F32, tag="mm", bufs=2,
                          name=f"pmo_{fc}_{tt}")
            for dc in range(DC):
                nc.tensor.matmul(
                    pm[:], wo_sb[dc][:, fc * P : (fc + 1) * P],
                    attnT[dc][:, sl], start=(dc == 0), stop=(dc == DC - 1),
                )
            nc.vector.scalar_tensor_tensor(
                x2T[fc][:, sl], pm[:], gcol(V_BO, fc),
                xres[:].bitcast(F32),
                ALU.add, ALU.add,
            )

        def w1_unit(tt, hc):
            sl = slice(tt * 512, (tt + 1) * 512)
            w1t = w1_p.tile([P, DC, P], BF16, tag="w1",
                            name=f"w1_{tt}_{hc}")
            nc.sync.dma_start(w1t[:], w1[:, hc, :, :])
            ph = ps6.tile([P, 512], F32, tag="mm", bufs=2,
                          name=f"ph1_{tt}_{hc}")
            for dc in range(DC):
                nc.tensor.matmul(
                    ph[:], w1t[:, dc, :],
                    xhat2T[dc][:, sl],
                    start=(dc == 0), stop=(dc == DC - 1),
                )
            ht = h1_p.tile([P, 512], BF16, tag="h1", name=f"h1_{tt}_{hc}")
            nc.scalar.activation(ht[:], ph[:], AF.Gelu,
                                 bias=b1t[:, hc : hc + 1])
            h1[tt, hc] = ht

        x2_load = lambda dc, tt: x2T[dc][:, tt * 512 : (tt + 1) * 512]
        for fc in range(DC):
            outproj_unit(0, fc)
        ln_chunk(ps6, x2_load, xhat2T, 0, "xhat2", False, False)
        # W1(tt0) PE work hides outproj(tt1)/LN2(tt1) DVE+ActE work;
        # the last 6 units are held back to cover LN2(tt1)'s DVE chain
        HB = 6
        for fc in range(DC):
            outproj_unit(1, fc)
            for hc in range(4 * fc, 4 * fc + 4):
                if hc < NHC - HB:
                    w1_unit(0, hc)
        ln_chunk(ps6, x2_load, xhat2T, 1, "xhat2", False, False)
        for hc in range(NHC - HB, NHC):
            w1_unit(0, hc)
        for hc in range(NHC):
            w1_unit(1, hc)
        if DBG:
            dbg_dump(x2T[0][:], d_x2T, TQ)
        w1_cm.__exit__(None, None, None)
        xhat2_cm.__exit__(None, None, None)
        xin6_cm.__exit__(None, None, None)
        attnT_cm.__exit__(None, None, None)
        wo_cm.__exit__(None, None, None)
        ps6_cm.__exit__(None, None, None)

        # ================= phase 8b: MLP down-proj, 8-bank pass ========
        ps8_cm, ps8 = _psum(tc, "ps8")
        w2_cm, w2_p = _pool(tc, "w2s", 5)
        out_cm, out_p = _pool(tc, "ostage", 4)
        for tt in range(TQ // 512):
            sl = slice(tt * 512, (tt + 1) * 512)
            pms = ps8.tile([P, DC, 512], F32, tag="mlp8", bufs=1,
                           name=f"pmh2_{tt}")
            for hc in range(NHC):
                w2t = w2_p.tile([P, D], BF16, tag="w2",
                                name=f"w2_{tt}_{hc}")
                nc.sync.dma_start(w2t[:], w2[hc * P : (hc + 1) * P, :])
                for fc in range(DC):
                    nc.tensor.matmul(
                        pms[:, fc, :], w2t[:, fc * P : (fc + 1) * P],
                        h1[tt, hc][:], start=(hc == 0), stop=(hc == NHC - 1),
                    )
            for fc in range(DC):
                ot = out_p.tile([P, 512], F32, tag="ostage",
                                name=f"ot_{tt}_{fc}")
                nc.vector.scalar_tensor_tensor(
                    ot[:], pms[:, fc, :], gcol(V_B2, fc), x2T[fc][:, sl],
                    ALU.add, ALU.add,
                )
                nc.sync.dma_start(outT[fc * P : (fc + 1) * P, sl], ot[:])

        out_cm.__exit__(None, None, None)
        w2_cm.__exit__(None, None, None)
        h1_cm.__exit__(None, None, None)
        x2_cm.__exit__(None, None, None)
        ps8_cm.__exit__(None, None, None)
        stat_cm.__exit__(None, None, None)
        work_cm.__exit__(None, None, None)
        const_cm.__exit__(None, None, None)

    nc.compile()
    return nc


# old feature index (within a 64-dim head block) at each new position:
# pairs (j, j+32) become adjacent (2j, 2j+1)
OLD_OF_NEW = np.array([j // 2 if j % 2 == 0 else j // 2 + 32
                       for j in range(HD)])


def _perm_cols(a):
    """Permute the last dim (64-multiple) per 64-feature head block."""
    a = np.asarray(a, np.float32)
    shp = a.shape
    nb = shp[-1] // HD
    a = a.reshape(shp[:-1] + (nb, HD))
    a = a[..., OLD_OF_NEW]
    return a.reshape(shp)


def _pairswap(a):
    """Swap even/odd positions of the last dim."""
    a = np.asarray(a, np.float32)
    shp = a.shape
    a = a.reshape(shp[:-1] + (shp[-1] // 2, 2))
    a = a[..., ::-1]
    return np.ascontiguousarray(a.reshape(shp))


def _col8(v):
    return np.ascontiguousarray(
        np.asarray(v, np.float32).reshape(DC, P).T.astype(np.float32))


def make_in_maps(inputs):
    x = np.asarray(inputs["x"], np.float32)
    context = np.asarray(inputs["context"], np.float32)
    cos = np.asarray(inputs["rope_cos"], np.float32).reshape(L, HD)
    sin = np.asarray(inputs["rope_sin"], np.float32).reshape(L, HD)

    bf = lambda a: np.ascontiguousarray(np.asarray(a, np.float32)).astype(
        ml_dtypes.bfloat16)
    Wq = np.asarray(inputs["Wq"], np.float32)
    Wkv = np.asarray(inputs["Wkv"], np.float32)
    W1 = np.asarray(inputs["W1"], np.float32)
    g_q = np.asarray(inputs["g_q"], np.float32)
    be_q = np.asarray(inputs["be_q"], np.float32)
    g_kv = np.asarray(inputs["g_kv"], np.float32)
    be_kv = np.asarray(inputs["be_kv"], np.float32)
    g_ffn = np.asarray(inputs["g_ffn"], np.float32)
    be_ffn = np.asarray(inputs["be_ffn"], np.float32)

    # fold LN gamma/beta into weights/biases
    Wq_f = g_q[:, None] * Wq
    bq_f = be_q @ Wq + np.asarray(inputs["bq"], np.float32)
    Wk_f = g_kv[:, None] * Wkv[:, :D]
    bk_f = be_kv @ Wkv[:, :D] + np.asarray(inputs["bkv"], np.float32)[:D]
    Wv_f = g_kv[:, None] * Wkv[:, D:]
    bv_f = be_kv @ Wkv[:, D:] + np.asarray(inputs["bkv"], np.float32)[D:]
    W1_f = g_ffn[:, None] * W1
    b1_f = be_ffn @ W1 + np.asarray(inputs["b1"], np.float32)

    # rope pair permutation on q/k output features
    Wq_p = _perm_cols(Wq_f)
    bq_p = _perm_cols(bq_f)
    Wk_p = _perm_cols(Wk_f)
    bk_p = _perm_cols(bk_f)

    # q/k/v weights: fp8e4m3 packed [p, dc, m] = W[dc*128+p, m]
    f8 = lambda a: np.ascontiguousarray(
        np.asarray(a, np.float32).reshape(DC, P, D).transpose(1, 0, 2)
    ).astype(ml_dtypes.float8_e4m3fn)
    wq_b = f8(Wq_p)
    wk_b = f8(Wk_p)
    wv_b = f8(Wv_f)
    wo_b = bf(inputs["Wo"])
    # w1 packed [p, hc, dc, j] = W1[dc*128+p, hc*128+j]
    w1_b = bf(np.ascontiguousarray(
        W1_f.reshape(DC, P, NHC, P).transpose(1, 2, 0, 3)))
    w2_b = bf(inputs["W2"])

    vecs = np.stack(
        [_col8(bq_p), _col8(bk_p),
         _col8(inputs["bo"]), _col8(inputs["b2"]),
         _col8(_pairswap(bq_p)), _col8(_pairswap(bk_p))],
        axis=1,
    )  # [128, 6, 8]
    vecs = np.ascontiguousarray(vecs)
    b1t = np.ascontiguousarray(b1_f.reshape(NHC, P).T)
    bvrow = np.ascontiguousarray(bv_f.reshape(1, D))

    # rope tables in permuted feature space:
    # cosP[n] = cos[old_of_new[n]]; sinE[2j] = -sin[j], sinE[2j+1] = sin[j+32]
    cosP = cos[:, OLD_OF_NEW]                        # [L, 64]
    sinP = sin[:, OLD_OF_NEW]
    sinE = sinP.copy()
    sinE[:, 0::2] = -sinE[:, 0::2]
    cosT = cosP.T                                    # [64, L]
    sinT = sinE.T
    cosk_full = np.ascontiguousarray(np.concatenate([cosT, cosT], 0)).astype(
        ml_dtypes.bfloat16)
    sink_full = np.ascontiguousarray(np.concatenate([sinT, sinT], 0)).astype(
        ml_dtypes.bfloat16)

    in_maps = []
    for c in range(NCORES):
        b, hf = c // 2, c % 2
        tsl = slice(hf * TQ, (hf + 1) * TQ)
        in_maps.append({
            "xT": np.ascontiguousarray(x[b, tsl, :].T),
            "ctxT": np.ascontiguousarray(context[b].T),
            "cosq": np.ascontiguousarray(cosk_full[:, tsl]),
            "sinq": np.ascontiguousarray(sink_full[:, tsl]),
            "cosk": cosk_full,
            "sink": sink_full,
            "wq": wq_b, "wk": wk_b, "wv": wv_b, "wo": wo_b,
            "w1": w1_b, "w2": w2_b,
            "vecs": vecs, "b1t": b1t, "bvrow": bvrow,
            "onesr": np.ones((P, 1), np.float32),
        })
    return in_maps


def kernel(**inputs) -> np.ndarray:
    global _CACHED_NC
    if _CACHED_NC is None:
        _CACHED_NC = build_nc()
    nc = _CACHED_NC
    in_maps = make_in_maps(inputs)
    res = run_bass_kernel_spmd(nc, in_maps, core_ids=list(range(NCORES)))
    out = np.empty((B, L, D), np.float32)
    for c in range(NCORES):
        b, hf = c // 2, c % 2
        out[b, hf * TQ : (hf + 1) * TQ, :] = res.results[c]["outT"].T
    return out



# revision 69
# speedup vs baseline: 1.0570x; 1.0233x over previous
"""Cross-attention transformer block on 8 TRN2 NeuronCores.

Sharding: 8 cores = 4 batches x 2 sequence-halves. Core c handles batch
b = c//2, query tokens [hf*1024, (hf+1)*1024) with hf = c%2. Each core
computes the FULL kv projection for its batch (duplicated across the 2
cores of a batch) so no collectives are needed.

Feature-major layout ([feature, token]) so matmuls contract over the
partition dim with natural weight layouts. Optimizations on top of the
v2 baseline (915us -> ~845us on the fast clock state):
  - q/k/v projections run fp8e4m3 with DoubleRow (256-deep contraction
    per pass, ~1.8x PE); LN outputs are emitted as [128, 2, T] fp8
    pair-tiles so DoubleRow slices them directly. v (vext) stays fp8
    as the U-matmul stationary. W1/W2/Wo remain bf16: fp8 there pushed
    rel err to ~2e-2 (the MLP path has no averaging to wash out
    quantization noise; attention does).
  - LN normalize and RoPE run on bf16 DVE 2x mode; ActE evicts psum
    to bf16 first (ActE is idle in those phases). Rope tables bf16.
  - All reciprocals use the custom-DVE reciprocal_approx_fast (~5x
    over InstReciprocal, which cost 3.3us per [1,512] row). NOTE: the
    custom op only honors base_partition 0 - softmax denominators are
    first copied from psum partition 64 to a partition-0 SBUF row.
  - Emission interleaves LN(x) chunks into the k/v unit stream so the
    LN DVE/ActE work hides under projection PE work; q is emitted last
    so attention (which needs qT[fc] ascending) can start early.
  - Attention: scores as two concurrent K=64 row-tiled matmuls; exp
    [128,1024] per head-pair on ActE (attention is ActE-bound: 284us
    of exp at 1 elem/lane/cycle is the phase floor); U with an
    appended ones-column (M=65) accumulates both U and the softmax
    denominator.
  - Weight DMA prefetch is emitted behind the first input chunk's
    loads; W1/W2 stream with 4-5 deep pools.
"""

import numpy as np
import ml_dtypes

import concourse.bass as bass
import concourse.bacc as bacc
import concourse.mybir as mybir
import concourse.tile as tile
from concourse.bass_utils import run_bass_kernel_spmd

F32 = mybir.dt.float32
F32R = mybir.dt.float32r
BF16 = mybir.dt.bfloat16
FP8 = mybir.dt.float8e4
AF = mybir.ActivationFunctionType
ALU = mybir.AluOpType
DR = mybir.MatmulPerfMode.DoubleRow

B, L, D, H, HD = 4, 2048, 1024, 16, 64
TQ = 1024          # query tokens per core
TK = 2048          # kv tokens per core
HID = 4 * D
NCORES = 8
P = 128
DC = D // P        # 8 feature chunks
KC = TK // P       # 16 kv-token chunks
NHC = HID // P     # 32 hidden chunks
EPS = 1e-5

# vecs[:, i, :] packing indices
(V_BQ, V_BK, V_BO, V_B2, V_BQR, V_BKR) = range(6)

PAIRSWAP_MASK = [i + 1 if i % 2 == 0 else i - 1 for i in range(32)]

_CACHED_NC = None


def _pool(tc, name, bufs, side="left"):
    cm = tc.tile_pool(name=name, bufs=bufs, side=side)
    return cm, cm.__enter__()


def _psum(tc, name):
    cm = tc.tile_pool(name=name, bufs=1, space="PSUM")
    return cm, cm.__enter__()


def build_nc():
    nc = bacc.Bacc("TRN2", debug=False, num_devices=NCORES)

    xT = nc.declare_dram_parameter("xT", [D, TQ], F32R, False).ap()
    ctxT = nc.declare_dram_parameter("ctxT", [D, TK], F32R, False).ap()
    cosq = nc.declare_dram_parameter("cosq", [P, TQ], BF16, False).ap()
    sinq = nc.declare_dram_parameter("sinq", [P, TQ], BF16, False).ap()
    cosk = nc.declare_dram_parameter("cosk", [P, TK], BF16, False).ap()
    sink = nc.declare_dram_parameter("sink", [P, TK], BF16, False).ap()
    # q/k/v weights packed [p, dc, m] = W[dc*128+p, m], fp8 for DoubleRow
    wq = nc.declare_dram_parameter("wq", [P, DC, D], FP8, False).ap()
    wk = nc.declare_dram_parameter("wk", [P, DC, D], FP8, False).ap()
    wv = nc.declare_dram_parameter("wv", [P, DC, D], FP8, False).ap()
    wo = nc.declare_dram_parameter("wo", [D, D], BF16, False).ap()
    # w1 packed [p, hc, dc, j] = W1[dc*128+p, hc*128+j]
    w1 = nc.declare_dram_parameter("w1", [P, NHC, DC, P], BF16, False).ap()
    w2 = nc.declare_dram_parameter("w2", [HID, D], BF16, False).ap()
    vecs_d = nc.declare_dram_parameter("vecs", [P, 6, DC], F32, False).ap()
    b1t_d = nc.declare_dram_parameter("b1t", [P, NHC], F32, False).ap()
    bvrow_d = nc.declare_dram_parameter("bvrow", [1, D], F32, False).ap()
    onesr_d = nc.declare_dram_parameter("onesr", [P, 1], F32R, False).ap()
    outT = nc.declare_dram_parameter("outT", [D, TQ], F32, True).ap()
    import os
    DBG = os.environ.get("KDBG", "0") == "1"
    if DBG:
        d_chat = nc.declare_dram_parameter("d_chat", [P, TK], F32, True).ap()
        d_xhat = nc.declare_dram_parameter("d_xhat", [P, TQ], F32, True).ap()
        d_qT = nc.declare_dram_parameter("d_qT", [P, TQ], F32, True).ap()
        d_kT = nc.declare_dram_parameter("d_kT", [P, TK], F32, True).ap()
        d_attnT = nc.declare_dram_parameter("d_attnT", [P, TQ], F32, True).ap()
        d_x2T = nc.declare_dram_parameter("d_x2T", [P, TQ], F32, True).ap()
        d_pu = nc.declare_dram_parameter("d_pu", [P, 512], F32, True).ap()
        d_rcp = nc.declare_dram_parameter("d_rcp", [1, 512], F32, True).ap()
        d_rb = nc.declare_dram_parameter("d_rb", [1, 512], F32, True).ap()

    with tile.TileContext(nc) as tc:
        const_cm, const = _pool(tc, "const", 1)
        work_cm, work = _pool(tc, "work", 8)       # f32 [128,512] scratch
        stat_cm, stat = _pool(tc, "stat", 4)

        # ---- constants ----
        vecs = const.tile([P, 6, DC], F32, tag="vecs")
        nc.sync.dma_start(vecs[:], vecs_d)
        b1t = const.tile([P, NHC], F32, tag="b1t")
        nc.sync.dma_start(b1t[:], b1t_d)
        bvrow = const.tile([1, D], F32, tag="bvrow")
        nc.sync.dma_start(bvrow[:], bvrow_d)
        bvb = const.tile([P, D], F32, tag="bvb")
        nc.gpsimd.partition_broadcast(bvb[:], bvrow[:])
        onesP = const.tile([P, 1], F32, tag="onesP")
        nc.vector.memset(onesP[:], 1.0)
        onesPr = const.tile([P, 1], F32R, tag="onesPr")
        nc.sync.dma_start(onesPr[:], onesr_d)
        onesPb = const.tile([P, 1], BF16, tag="onesPb")
        nc.vector.memset(onesPb[:], 1.0)
        eps1 = const.tile([1, 1], F32, tag="eps1")
        nc.vector.memset(eps1[:], EPS)

        def scratch(name):
            return work.tile([P, 512], F32, tag="scratch", name=name)

        def gcol(idx, dc):
            return vecs[:, idx, dc : dc + 1]

        def ln_alloc(out_pool, out_tag, nt, paired):
            if paired:
                return [out_pool.tile([P, 2, nt], FP8, tag=out_tag,
                                      name=f"{out_tag}{i}")
                        for i in range(DC // 2)]
            return [out_pool.tile([P, nt], BF16, tag=out_tag,
                                  name=f"{out_tag}{i}") for i in range(DC)]

        def ln_chunk(ps, load_fn, outs, tt, out_tag, src_r, paired):
            """One 512-token LayerNorm chunk (gamma/beta folded host-side)."""
            if True:
                sl = slice(tt * 512, (tt + 1) * 512)
                raw = [load_fn(dc, tt) for dc in range(DC)]
                srcs = [r.bitcast(F32) if src_r else r for r in raw]
                pr_row = ps.tile([P, 512], F32, tag="row", bufs=1,
                                 name=f"lnrow_{out_tag}_{tt}")
                # sum on partition 0, sumsq on partition 32 (same bank)
                for dc in range(DC):
                    sq = work.tile([P, 512], BF16, tag="sq", bufs=2,
                                   name=f"sq_{out_tag}_{tt}_{dc}")
                    nc.scalar.square(sq[:], srcs[dc])
                    nc.tensor.matmul(
                        pr_row[0:1, :],
                        onesPr[:] if src_r else onesPb[:],
                        raw[dc],
                        start=(dc == 0), stop=(dc == DC - 1),
                    )
                    nc.tensor.matmul(
                        pr_row[32:33, :], onesPb[:],
                        sq[:],
                        start=(dc == 0), stop=(dc == DC - 1),
                    )
                st = stat.tile([1, 3, 512], F32, tag="stats", bufs=2,
                               name=f"st_{out_tag}_{tt}")
                mu, var, rs = (st[:, i, :] for i in range(3))
                nc.vector.tensor_scalar_mul(mu, pr_row[0:1, :], 1.0 / D)
                nc.vector.tensor_scalar_mul(rs, pr_row[32:33, :], 1.0 / D)
                nc.vector.tensor_mul(var, mu, mu)
                nc.vector.tensor_sub(var, rs, var)
                # rs <- sqrt(var+eps) then var <- 1/rs (fast approx)
                nc.scalar.activation(rs, var, AF.Sqrt, bias=eps1[:])
                nc.vector.reciprocal_approx_fast(var, rs)
                rs = var
                # bf16 stats rows -> bf16 broadcasts -> bf16 2x normalize
                stb = stat.tile([1, 2, 512], BF16, tag="statsb", bufs=1,
                                name=f"stb_{out_tag}_{tt}")
                mu_b, rs_b = stb[:, 0, :], stb[:, 1, :]
                nc.vector.tensor_copy(mu_b, mu)
                nc.vector.tensor_copy(rs_b, rs)
                mub = work.tile([P, 512], BF16, tag="mub", bufs=2,
                                name=f"mub_{out_tag}_{tt}")
                nc.gpsimd.partition_broadcast(mub[:], mu_b)
                rsb = work.tile([P, 512], BF16, tag="rsb", bufs=2,
                                name=f"rsb_{out_tag}_{tt}")
                nc.gpsimd.partition_broadcast(rsb[:], rs_b)
                for dc in range(DC):
                    # ActE evicts src to bf16 so both DVE ops run 2x mode
                    xb = work.tile([P, 512], BF16, tag="xb", bufs=2,
                                   name=f"xb_{out_tag}_{tt}_{dc}")
                    nc.scalar.activation(xb[:], srcs[dc], AF.Copy)
                    t = work.tile([P, 512], BF16, tag="lnt", bufs=2,
                                  name=f"lnt_{out_tag}_{tt}_{dc}")
                    nc.vector.tensor_sub(t[:], xb[:], mub[:])
                    dst = (outs[dc // 2][:, dc % 2, sl] if paired
                           else outs[dc][:, sl])
                    nc.vector.tensor_mul(dst, t[:], rsb[:])

        def ln_T(ps, load_fn, nt, out_pool, out_tag, src_r, paired=False):
            outs = ln_alloc(out_pool, out_tag, nt, paired)
            for tt in range(nt // 512):
                ln_chunk(ps, load_fn, outs, tt, out_tag, src_r, paired)
            return outs

        def rope_evict(psum, out_ap, cos_t, sin_t, sl, b_idx, b_rot_idx, fc):
            """out = (psum + b)*cosP + pairswap(psum + b)*sinE (bf16).

            bR = pairswap(b), so applying b on the ActE eviction (bias is
            per-partition there, free) makes the shuffled copy carry bR
            automatically; both DVE multiplies then run as bf16 2x TT
            (STT has no 2x uop and costs ~745ns vs 345ns for TT).
            """
            pb = work.tile([P, 512], BF16, tag="ropePb", bufs=2,
                           name=f"ropeP_{b_idx}_{fc}_{sl.start}")
            nc.scalar.activation(pb[:], psum[:], AF.Identity,
                                 bias=gcol(b_idx, fc))
            sh = work.tile([P, 512], BF16, tag="ropeSh", bufs=2,
                           name=f"ropeS_{b_idx}_{fc}_{sl.start}")
            nc.vector.stream_shuffle(sh[:], pb[:], PAIRSWAP_MASK)
            t = work.tile([P, 512], BF16, tag="ropeT", bufs=2,
                          name=f"ropeA_{b_idx}_{fc}_{sl.start}")
            nc.vector.tensor_mul(t[:], pb[:], cos_t[:, sl])
            t2 = work.tile([P, 512], BF16, tag="ropeT", bufs=2,
                           name=f"ropeB_{b_idx}_{fc}_{sl.start}")
            nc.vector.tensor_mul(t2[:], sh[:], sin_t[:, sl])
            nc.vector.tensor_add(out_ap, t[:], t2[:])

        def dram_loader(pool, dram_ap, tag):
            def load(dc, tt):
                t = pool.tile([P, 512], F32R, tag=tag,
                              name=f"{tag}_{dc}_{tt}")
                nc.sync.dma_start(
                    t[:], dram_ap[dc * P : (dc + 1) * P,
                                  tt * 512 : (tt + 1) * 512])
                return t[:]
            return load

        ps1_cm, ps1 = _psum(tc, "ps1")

        def dbg_dump(src_ap, dram_ap, n):
            if not DBG:
                return
            for c in range(n // 512):
                s = work.tile([P, 512], F32, tag="dbgs", bufs=2,
                              name=f"dbg_{dram_ap}_{c}")
                nc.vector.tensor_copy(
                    s[:], src_ap[:, c * 512 : (c + 1) * 512])
                nc.sync.dma_start(
                    dram_ap[:, c * 512 : (c + 1) * 512], s[:])

        # ====== front: LN(ctx), then {LN(x) | k | v | q} interleaved ======
        # weights prefetched up front so DMA hides under LN compute
        w_cm, w_p = _pool(tc, "wqkv", 8, side="right")
        chat_cm, chat_p = _pool(tc, "chat", DC, side="right")
        # left-stack order chosen for LIFO exits:
        # qT/kT/vext (die after attention) below xhat (dies after q),
        # then cin (dies after ctx-LN), then xin (dies after interleave)
        qT_cm, qT_p = _pool(tc, "qT", DC)
        kT_cm, kT_p = _pool(tc, "kT", DC)
        vext_cm, vext_p = _pool(tc, "vext", KC)
        xhat_cm, xhat_p = _pool(tc, "xhat", DC)

        cin_cm, cin_p = _pool(tc, "cin", 12)
        chatT = ln_alloc(chat_p, "chat", TK, paired=True)
        cin_load = dram_loader(cin_p, ctxT, "cin")
        wk_sb = wv_sb = None
        for tt in range(TK // 512):
            ln_chunk(ps1, cin_load, chatT, tt, "chat", True, True)
            if tt == 0:
                # weight prefetch behind the first input chunk's DMAs
                wk_sb = w_p.tile([P, DC, D], FP8, tag="w8", bufs=2,
                                 name="wk8")
                nc.sync.dma_start(wk_sb[:], wk)
                wv_sb = w_p.tile([P, DC, D], FP8, tag="w8", bufs=2,
                                 name="wv8")
                nc.sync.dma_start(wv_sb[:], wv)
        cin_cm.__exit__(None, None, None)

        ropek_cm, ropek_p = _pool(tc, "ropek", 1, side="right")
        cosk_t = ropek_p.tile([P, TK], BF16, tag="cosk")
        nc.sync.dma_start(cosk_t[:], cosk)
        sink_t = ropek_p.tile([P, TK], BF16, tag="sink")
        nc.sync.dma_start(sink_t[:], sink)

        xin_cm, xin_p = _pool(tc, "xin", 8)
        xhatT = ln_alloc(xhat_p, "xhat", TQ, paired=True)
        xin_load = dram_loader(xin_p, xT, "xin")
        kT = [kT_p.tile([P, TK], BF16, tag="kT", name=f"kT{i}")
              for i in range(DC)]
        vext = []
        for kc in range(KC):
            vt = vext_p.tile([P, H, HD + 1], FP8, tag="vext",
                             name=f"vext{kc}")
            nc.vector.memset(vt[:, :, HD : HD + 1], 1.0)
            vext.append(vt)

        def k_unit(fc):
            for tt in range(TK // 512):
                sl = slice(tt * 512, (tt + 1) * 512)
                pm = ps1.tile([P, 512], F32, tag="mm", bufs=4,
                              name=f"pmk_{fc}_{tt}")
                for i in range(DC // 2):
                    nc.tensor.matmul(
                        pm[:],
                        wk_sb[:, 2 * i : 2 * i + 2, fc * P : (fc + 1) * P],
                        chatT[i][:, :, sl],
                        start=(i == 0), stop=(i == DC // 2 - 1),
                        perf_mode=DR,
                    )
                rope_evict(pm, kT[fc][:, sl], cosk_t, sink_t, sl,
                           V_BK, V_BKR, fc)

        def v_unit(kc):
            for f2 in range(2):
                pm = ps1.tile([P, 512], F32, tag="mm", bufs=4,
                              name=f"pmv_{kc}_{f2}")
                for i in range(DC // 2):
                    nc.tensor.matmul(
                        pm[:], chatT[i][:, :, kc * P : (kc + 1) * P],
                        wv_sb[:, 2 * i : 2 * i + 2,
                              f2 * 512 : (f2 + 1) * 512],
                        start=(i == 0), stop=(i == DC // 2 - 1),
                        perf_mode=DR,
                    )
                nc.vector.tensor_add(
                    vext[kc][:, f2 * 8 : (f2 + 1) * 8, 0:HD],
                    pm[:].rearrange("p (h d) -> p h d", d=HD),
                    bvb[:, f2 * 512 : (f2 + 1) * 512].rearrange(
                        "p (h d) -> p h d", d=HD),
                )

        def q_unit(fc):
            for tt in range(TQ // 512):
                sl = slice(tt * 512, (tt + 1) * 512)
                pm = ps1.tile([P, 512], F32, tag="mm", bufs=4,
                              name=f"pmq_{fc}_{tt}")
                for i in range(DC // 2):
                    nc.tensor.matmul(
                        pm[:],
                        wq_sb[:, 2 * i : 2 * i + 2, fc * P : (fc + 1) * P],
                        xhatT[i][:, :, sl],
                        start=(i == 0), stop=(i == DC // 2 - 1),
                        perf_mode=DR,
                    )
                rope_evict(pm, qT[fc][:, sl], cosq_t, sinq_t, sl,
                           V_BQ, V_BQR, fc)

        # interleave: PE-heavy k/v units hide LN(x)/rope-k DVE+ActE work
        for fc in range(DC):
            k_unit(fc)
            v_unit(2 * fc)
            v_unit(2 * fc + 1)
            if fc < TQ // 512:
                ln_chunk(ps1, xin_load, xhatT, fc, "xhat", True, True)
        xin_cm.__exit__(None, None, None)
        ropek_cm.__exit__(None, None, None)
        chat_cm.__exit__(None, None, None)

        # q last (xin/ropek space freed)
        wqp_cm, wqp_p = _pool(tc, "wqp", 1, side="right")
        wq_sb = wqp_p.tile([P, DC, D], FP8, tag="wq8", name="wq8")
        nc.sync.dma_start(wq_sb[:], wq)
        ropeq_cm, ropeq_p = _pool(tc, "ropeq", 1, side="right")
        cosq_t = ropeq_p.tile([P, TQ], BF16, tag="cosq")
        nc.sync.dma_start(cosq_t[:], cosq)
        sinq_t = ropeq_p.tile([P, TQ], BF16, tag="sinq")
        nc.sync.dma_start(sinq_t[:], sinq)
        qT = [qT_p.tile([P, TQ], BF16, tag="qT", name=f"qT{i}")
              for i in range(DC)]
        for fc in range(DC):
            q_unit(fc)
        ropeq_cm.__exit__(None, None, None)
        wqp_cm.__exit__(None, None, None)
        xhat_cm.__exit__(None, None, None)
        w_cm.__exit__(None, None, None)
        ps1_cm.__exit__(None, None, None)

        # ================= phase 5: attention =================
        # Per (tt, head-pair): 16 kc steps. Each kc: two concurrent K=64
        # score matmuls (PE row tiles 0/64) into a 2-bank psum pair-tile,
        # one [128,1024] exp, two U accumulations (M=65, ones-column
        # denominator). Head pairs processed two at a time so softmax
        # reciprocals batch 4 heads per DVE call.
        ps5_cm, ps5 = _psum(tc, "ps5")
        # wo prefetched here so its DMA hides under attention
        wo_cm, wo_p = _pool(tc, "wo", DC, side="right")
        wo_sb = []
        for dc in range(DC):
            wt = wo_p.tile([P, D], BF16, tag="wo", name=f"wo{dc}")
            nc.sync.dma_start(wt[:], wo[dc * P : (dc + 1) * P, :])
            wo_sb.append(wt)
        attnT_cm, attnT_p = _pool(tc, "attnT", DC, side="right")
        e_cm, e_p = _pool(tc, "epool", 4)
        attnT = [attnT_p.tile([P, TQ], BF16, tag="attnT", name=f"attnT{i}")
                 for i in range(DC)]
        for tt in range(TQ // 512):
            sl = slice(tt * 512, (tt + 1) * 512)
            for hp2 in range(4):       # pairs of head-pairs
                pus = []
                for i in range(2):     # head pair index within group
                    fc = hp2 * 2 + i
                    pu = ps5.tile([P, 2, 512], F32, tag="u", bufs=2,
                                  name=f"pu_{tt}_{fc}")
                    pus.append(pu)
                    for kc in range(KC):
                        psc = ps5.tile([P, 2, 512], F32, tag="sc", bufs=2,
                                       name=f"psc_{tt}_{fc}_{kc}")
                        for j in range(2):   # head row-halves, concurrent
                            hb = j * HD
                            nc.tensor.matmul(
                                psc[:, j, :],
                                kT[fc][hb : hb + HD, kc * P : (kc + 1) * P],
                                qT[fc][hb : hb + HD, sl],
                                start=True, stop=True,
                            )
                        e = e_p.tile([P, 2, 512], BF16, tag="e",
                                     name=f"e_{tt}_{fc}_{kc}")
                        nc.scalar.activation(e[:], psc[:], AF.Exp, scale=0.125)
                        for j in range(2):
                            nc.tensor.matmul(
                                pu[0 : HD + 1, j, :],
                                vext[kc][:, fc * 2 + j, :],
                                e[:, j, :],
                                start=(kc == 0), stop=(kc == KC - 1),
                            )
                # softmax epilogue: copy den row to a partition-0 SBUF tile
                # (the custom-DVE reciprocal only honors base_partition 0),
                # then one fast-approx reciprocal for both heads
                for i in range(2):
                    fc = hp2 * 2 + i
                    den0 = stat.tile([1, 2, 512], F32, tag="den0", bufs=1,
                                     name=f"den0_{tt}_{fc}")
                    nc.vector.tensor_copy(
                        den0[:], pus[i][HD : HD + 1, :, :])
                    rcp = stat.tile([1, 2, 512], F32, tag="rcp", bufs=2,
                                    name=f"rcp_{tt}_{fc}")
                    nc.vector.reciprocal_approx_fast(rcp[:], den0[:])
                    for j in range(2):
                        hb = j * HD
                        rb = work.tile([HD, 512], F32, tag="rb", bufs=2,
                                       name=f"rb_{tt}_{fc}_{j}")
                        nc.gpsimd.partition_broadcast(rb[:], rcp[:, j, :])
                        nc.vector.tensor_mul(
                            attnT[fc][hb : hb + HD, sl],
                            pus[i][0:HD, j, :], rb[:])
        if DBG:
            dbg_dump(attnT[0][:], d_attnT, TQ)
        e_cm.__exit__(None, None, None)
        vext_cm.__exit__(None, None, None)
        kT_cm.__exit__(None, None, None)
        qT_cm.__exit__(None, None, None)
        ps5_cm.__exit__(None, None, None)

        # ================= phase 6: x2^T = Wo^T attn + x^T + bo ========
        ps6_cm, ps6 = _psum(tc, "ps6")
        x2_cm, x2_p = _pool(tc, "x2", DC)
        xin6_cm, xin6_p = _pool(tc, "xin6", 4, side="right")
        xhat2_cm, xhat2_p = _pool(tc, "xhat2", DC, side="right")
        h1_cm, h1_p = _pool(tc, "h1", 2 * NHC)
        w1_cm, w1_p = _pool(tc, "w1s", 4)
        x2T = [x2_p.tile([P, TQ], BF16, tag="x2", name=f"x2T{i}")
               for i in range(DC)]
        xhat2T = ln_alloc(xhat2_p, "xhat2", TQ, False)
        h1 = {}

        def outproj_unit(tt, fc):
            sl = slice(tt * 512, (tt + 1) * 512)
            xres = xin6_p.tile([P, 512], F32R, tag="xin6",
                               name=f"xres_{fc}_{tt}")
            nc.sync.dma_start(xres[:], xT[fc * P : (fc + 1) * P, sl])
            pm = ps6.tile([P, 512], F32, tag="mm", bufs=4,
                          name=f"pmo_{fc}_{tt}")
            for dc in range(DC):
                nc.tensor.matmul(
                    pm[:], wo_sb[dc][:, fc * P : (fc + 1) * P],
                    attnT[dc][:, sl], start=(dc == 0), stop=(dc == DC - 1),
                )
            nc.vector.scalar_tensor_tensor(
                x2T[fc][:, sl], pm[:], gcol(V_BO, fc),
                xres[:].bitcast(F32),
                ALU.add, ALU.add,
            )

        def w1_unit(tt, hc):
            sl = slice(tt * 512, (tt + 1) * 512)
            w1t = w1_p.tile([P, DC, P], BF16, tag="w1",
                            name=f"w1_{tt}_{hc}")
            nc.sync.dma_start(w1t[:], w1[:, hc, :, :])
            ph = ps6.tile([P, 512], F32, tag="mm", bufs=4,
                          name=f"ph1_{tt}_{hc}")
            for dc in range(DC):
                nc.tensor.matmul(
                    ph[:], w1t[:, dc, :],
                    xhat2T[dc][:, sl],
                    start=(dc == 0), stop=(dc == DC - 1),
                )
            ht = h1_p.tile([P, 512], BF16, tag="h1", name=f"h1_{tt}_{hc}")
            nc.scalar.activation(ht[:], ph[:], AF.Gelu,
                                 bias=b1t[:, hc : hc + 1])
            h1[tt, hc] = ht

        x2_load = lambda dc, tt: x2T[dc][:, tt * 512 : (tt + 1) * 512]
        for fc in range(DC):
            outproj_unit(0, fc)
        ln_chunk(ps6, x2_load, xhat2T, 0, "xhat2", False, False)
        # W1(tt0) PE work hides outproj(tt1)/LN2(tt1) DVE+ActE work;
        # the last 6 units are held back to cover LN2(tt1)'s DVE chain
        HB = 6
        for fc in range(DC):
            outproj_unit(1, fc)
            for hc in range(4 * fc, 4 * fc + 4):
                if hc < NHC - HB:
                    w1_unit(0, hc)
        ln_chunk(ps6, x2_load, xhat2T, 1, "xhat2", False, False)
        for hc in range(NHC - HB, NHC):
            w1_unit(0, hc)
        for hc in range(NHC):
            w1_unit(1, hc)
        if DBG:
            dbg_dump(x2T[0][:], d_x2T, TQ)
        w1_cm.__exit__(None, None, None)
        xhat2_cm.__exit__(None, None, None)
        xin6_cm.__exit__(None, None, None)
        attnT_cm.__exit__(None, None, None)
        wo_cm.__exit__(None, None, None)
        ps6_cm.__exit__(None, None, None)

        # ================= phase 8b: MLP down-proj, 8-bank pass ========
        ps8_cm, ps8 = _psum(tc, "ps8")
        w2_cm, w2_p = _pool(tc, "w2s", 5)
        out_cm, out_p = _pool(tc, "ostage", 4)
        for tt in range(TQ // 512):
            sl = slice(tt * 512, (tt + 1) * 512)
            pms = ps8.tile([P, DC, 512], F32, tag="mlp8", bufs=1,
                           name=f"pmh2_{tt}")
            for hc in range(NHC):
                w2t = w2_p.tile([P, D], BF16, tag="w2",
                                name=f"w2_{tt}_{hc}")
                nc.sync.dma_start(w2t[:], w2[hc * P : (hc + 1) * P, :])
                for fc in range(DC):
                    nc.tensor.matmul(
                        pms[:, fc, :], w2t[:, fc * P : (fc + 1) * P],
                        h1[tt, hc][:], start=(hc == 0), stop=(hc == NHC - 1),
                    )
            for fc in range(DC):
                ot = out_p.tile([P, 512], F32, tag="ostage",
                                name=f"ot_{tt}_{fc}")
                nc.vector.scalar_tensor_tensor(
                    ot[:], pms[:, fc, :], gcol(V_B2, fc), x2T[fc][:, sl],
                    ALU.add, ALU.add,
                )
                nc.sync.dma_start(outT[fc * P : (fc + 1) * P, sl], ot[:])

        out_cm.__exit__(None, None, None)
        w2_cm.__exit__(None, None, None)
        h1_cm.__exit__(None, None, None)
        x2_cm.__exit__(None, None, None)
        ps8_cm.__exit__(None, None, None)
        stat_cm.__exit__(None, None, None)
        work_cm.__exit__(None, None, None)
        const_cm.__exit__(None, None, None)

    nc.compile()
    return nc


# old feature index (within a 64-dim head block) at each new position:
# pairs (j, j+32) become adjacent (2j, 2j+1)
OLD_OF_NEW = np.array([j // 2 if j % 2 == 0 else j // 2 + 32
                       for j in range(HD)])


def _perm_cols(a):
    """Permute the last dim (64-multiple) per 64-feature head block."""
    a = np.asarray(a, np.float32)
    shp = a.shape
    nb = shp[-1] // HD
    a = a.reshape(shp[:-1] + (nb, HD))
    a = a[..., OLD_OF_NEW]
    return a.reshape(shp)


def _pairswap(a):
    """Swap even/odd positions of the last dim."""
    a = np.asarray(a, np.float32)
    shp = a.shape
    a = a.reshape(shp[:-1] + (shp[-1] // 2, 2))
    a = a[..., ::-1]
    return np.ascontiguousarray(a.reshape(shp))


def _col8(v):
    return np.ascontiguousarray(
        np.asarray(v, np.float32).reshape(DC, P).T.astype(np.float32))


def make_in_maps(inputs):
    x = np.asarray(inputs["x"], np.float32)
    context = np.asarray(inputs["context"], np.float32)
    cos = np.asarray(inputs["rope_cos"], np.float32).reshape(L, HD)
    sin = np.asarray(inputs["rope_sin"], np.float32).reshape(L, HD)

    bf = lambda a: np.ascontiguousarray(np.asarray(a, np.float32)).astype(
        ml_dtypes.bfloat16)
    Wq = np.asarray(inputs["Wq"], np.float32)
    Wkv = np.asarray(inputs["Wkv"], np.float32)
    W1 = np.asarray(inputs["W1"], np.float32)
    g_q = np.asarray(inputs["g_q"], np.float32)
    be_q = np.asarray(inputs["be_q"], np.float32)
    g_kv = np.asarray(inputs["g_kv"], np.float32)
    be_kv = np.asarray(inputs["be_kv"], np.float32)
    g_ffn = np.asarray(inputs["g_ffn"], np.float32)
    be_ffn = np.asarray(inputs["be_ffn"], np.float32)

    # fold LN gamma/beta into weights/biases
    Wq_f = g_q[:, None] * Wq
    bq_f = be_q @ Wq + np.asarray(inputs["bq"], np.float32)
    Wk_f = g_kv[:, None] * Wkv[:, :D]
    bk_f = be_kv @ Wkv[:, :D] + np.asarray(inputs["bkv"], np.float32)[:D]
    Wv_f = g_kv[:, None] * Wkv[:, D:]
    bv_f = be_kv @ Wkv[:, D:] + np.asarray(inputs["bkv"], np.float32)[D:]
    W1_f = g_ffn[:, None] * W1
    b1_f = be_ffn @ W1 + np.asarray(inputs["b1"], np.float32)

    # rope pair permutation on q/k output features
    Wq_p = _perm_cols(Wq_f)
    bq_p = _perm_cols(bq_f)
    Wk_p = _perm_cols(Wk_f)
    bk_p = _perm_cols(bk_f)

    # q/k/v weights: fp8e4m3 packed [p, dc, m] = W[dc*128+p, m]
    f8 = lambda a: np.ascontiguousarray(
        np.asarray(a, np.float32).reshape(DC, P, D).transpose(1, 0, 2)
    ).astype(ml_dtypes.float8_e4m3fn)
    wq_b = f8(Wq_p)
    wk_b = f8(Wk_p)
    wv_b = f8(Wv_f)
    wo_b = bf(inputs["Wo"])
    # w1 packed [p, hc, dc, j] = W1[dc*128+p, hc*128+j]
    w1_b = bf(np.ascontiguousarray(
        W1_f.reshape(DC, P, NHC, P).transpose(1, 2, 0, 3)))
    w2_b = bf(inputs["W2"])

    vecs = np.stack(
        [_col8(bq_p), _col8(bk_p),
         _col8(inputs["bo"]), _col8(inputs["b2"]),
         _col8(_pairswap(bq_p)), _col8(_pairswap(bk_p))],
        axis=1,
    )  # [128, 6, 8]
    vecs = np.ascontiguousarray(vecs)
    b1t = np.ascontiguousarray(b1_f.reshape(NHC, P).T)
    bvrow = np.ascontiguousarray(bv_f.reshape(1, D))

    # rope tables in permuted feature space:
    # cosP[n] = cos[old_of_new[n]]; sinE[2j] = -sin[j], sinE[2j+1] = sin[j+32]
    cosP = cos[:, OLD_OF_NEW]                        # [L, 64]
    sinP = sin[:, OLD_OF_NEW]
    sinE = sinP.copy()
    sinE[:, 0::2] = -sinE[:, 0::2]
    cosT = cosP.T                                    # [64, L]
    sinT = sinE.T
    cosk_full = np.ascontiguousarray(np.concatenate([cosT, cosT], 0)).astype(
        ml_dtypes.bfloat16)
    sink_full = np.ascontiguousarray(np.concatenate([sinT, sinT], 0)).astype(
        ml_dtypes.bfloat16)

    in_maps = []
    for c in range(NCORES):
        b, hf = c // 2, c % 2
        tsl = slice(hf * TQ, (hf + 1) * TQ)
        in_maps.append({
            "xT": np.ascontiguousarray(x[b, tsl, :].T),
            "ctxT": np.ascontiguousarray(context[b].T),
            "cosq": np.ascontiguousarray(cosk_full[:, tsl]),
            "sinq": np.ascontiguousarray(sink_full[:, tsl]),
            "cosk": cosk_full,
            "sink": sink_full,
            "wq": wq_b, "wk": wk_b, "wv": wv_b, "wo": wo_b,
            "w1": w1_b, "w2": w2_b,
            "vecs": vecs, "b1t": b1t, "bvrow": bvrow,
            "onesr": np.ones((P, 1), np.float32),
        })
    return in_maps


def kernel(**inputs) -> np.ndarray:
    global _CACHED_NC
    if _CACHED_NC is None:
        _CACHED_NC = build_nc()
    nc = _CACHED_NC
    in_maps = make_in_maps(inputs)
    res = run_bass_kernel_spmd(nc, in_maps, core_ids=list(range(NCORES)))
    out = np.empty((B, L, D), np.float32)
    for c in range(NCORES):
        b, hf = c // 2, c % 2
        out[b, hf * TQ : (hf + 1) * TQ, :] = res.results[c]["outT"].T
    return out



# revision 70
# speedup vs baseline: 1.0632x; 1.0059x over previous
"""Cross-attention transformer block on 8 TRN2 NeuronCores.

Sharding: 8 cores = 4 batches x 2 sequence-halves. Core c handles batch
b = c//2, query tokens [hf*1024, (hf+1)*1024) with hf = c%2. Each core
computes the FULL kv projection for its batch (duplicated across the 2
cores of a batch) so no collectives are needed.

Feature-major layout ([feature, token]) so matmuls contract over the
partition dim with natural weight layouts. Optimizations on top of the
v2 baseline (915us -> ~845us on the fast clock state):
  - q/k/v projections run fp8e4m3 with DoubleRow (256-deep contraction
    per pass, ~1.8x PE); LN outputs are emitted as [128, 2, T] fp8
    pair-tiles so DoubleRow slices them directly. v (vext) stays fp8
    as the U-matmul stationary. W1/W2/Wo remain bf16: fp8 there pushed
    rel err to ~2e-2 (the MLP path has no averaging to wash out
    quantization noise; attention does).
  - LN normalize and RoPE run on bf16 DVE 2x mode; ActE evicts psum
    to bf16 first (ActE is idle in those phases). Rope tables bf16.
  - All reciprocals use the custom-DVE reciprocal_approx_fast (~5x
    over InstReciprocal, which cost 3.3us per [1,512] row). NOTE: the
    custom op only honors base_partition 0 - softmax denominators are
    first copied from psum partition 64 to a partition-0 SBUF row.
  - Emission interleaves LN(x) chunks into the k/v unit stream so the
    LN DVE/ActE work hides under projection PE work; q is emitted last
    so attention (which needs qT[fc] ascending) can start early.
  - Attention: scores as two concurrent K=64 row-tiled matmuls; exp
    [128,1024] per head-pair on ActE (attention is ActE-bound: 284us
    of exp at 1 elem/lane/cycle is the phase floor); U with an
    appended ones-column (M=65) accumulates both U and the softmax
    denominator.
  - Weight DMA prefetch is emitted behind the first input chunk's
    loads; W1/W2 stream with 4-5 deep pools.
"""

import numpy as np
import ml_dtypes

import concourse.bass as bass
import concourse.bacc as bacc
import concourse.mybir as mybir
import concourse.tile as tile
from concourse.bass_utils import run_bass_kernel_spmd

F32 = mybir.dt.float32
F32R = mybir.dt.float32r
BF16 = mybir.dt.bfloat16
FP8 = mybir.dt.float8e4
AF = mybir.ActivationFunctionType
ALU = mybir.AluOpType
DR = mybir.MatmulPerfMode.DoubleRow

B, L, D, H, HD = 4, 2048, 1024, 16, 64
TQ = 1024          # query tokens per core
TK = 2048          # kv tokens per core
HID = 4 * D
NCORES = 8
P = 128
DC = D // P        # 8 feature chunks
KC = TK // P       # 16 kv-token chunks
NHC = HID // P     # 32 hidden chunks
EPS = 1e-5

# vecs[:, i, :] packing indices
(V_BQ, V_BK, V_BO, V_B2, V_BQR, V_BKR) = range(6)

PAIRSWAP_MASK = [i + 1 if i % 2 == 0 else i - 1 for i in range(32)]

_CACHED_NC = None


def _pool(tc, name, bufs, side="left"):
    cm = tc.tile_pool(name=name, bufs=bufs, side=side)
    return cm, cm.__enter__()


def _psum(tc, name):
    cm = tc.tile_pool(name=name, bufs=1, space="PSUM")
    return cm, cm.__enter__()


def build_nc():
    nc = bacc.Bacc("TRN2", debug=False, num_devices=NCORES)

    xT = nc.declare_dram_parameter("xT", [D, TQ], F32R, False).ap()
    ctxT = nc.declare_dram_parameter("ctxT", [D, TK], F32R, False).ap()
    cosq = nc.declare_dram_parameter("cosq", [P, TQ], BF16, False).ap()
    sinq = nc.declare_dram_parameter("sinq", [P, TQ], BF16, False).ap()
    cosk = nc.declare_dram_parameter("cosk", [P, TK], BF16, False).ap()
    sink = nc.declare_dram_parameter("sink", [P, TK], BF16, False).ap()
    # q/k/v weights packed [p, dc, m] = W[dc*128+p, m], fp8 for DoubleRow
    wq = nc.declare_dram_parameter("wq", [P, DC, D], FP8, False).ap()
    wk = nc.declare_dram_parameter("wk", [P, DC, D], FP8, False).ap()
    wv = nc.declare_dram_parameter("wv", [P, DC, D], FP8, False).ap()
    wo = nc.declare_dram_parameter("wo", [D, D], BF16, False).ap()
    # w1 packed [p, hc, dc, j] = W1[dc*128+p, hc*128+j]
    w1 = nc.declare_dram_parameter("w1", [P, NHC, DC, P], BF16, False).ap()
    w2 = nc.declare_dram_parameter("w2", [HID, D], BF16, False).ap()
    vecs_d = nc.declare_dram_parameter("vecs", [P, 6, DC], F32, False).ap()
    b1t_d = nc.declare_dram_parameter("b1t", [P, NHC], F32, False).ap()
    bvrow_d = nc.declare_dram_parameter("bvrow", [1, D], F32, False).ap()
    onesr_d = nc.declare_dram_parameter("onesr", [P, 1], F32R, False).ap()
    outT = nc.declare_dram_parameter("outT", [D, TQ], F32, True).ap()
    import os
    DBG = os.environ.get("KDBG", "0") == "1"
    if DBG:
        d_chat = nc.declare_dram_parameter("d_chat", [P, TK], F32, True).ap()
        d_xhat = nc.declare_dram_parameter("d_xhat", [P, TQ], F32, True).ap()
        d_qT = nc.declare_dram_parameter("d_qT", [P, TQ], F32, True).ap()
        d_kT = nc.declare_dram_parameter("d_kT", [P, TK], F32, True).ap()
        d_attnT = nc.declare_dram_parameter("d_attnT", [P, TQ], F32, True).ap()
        d_x2T = nc.declare_dram_parameter("d_x2T", [P, TQ], F32, True).ap()
        d_pu = nc.declare_dram_parameter("d_pu", [P, 512], F32, True).ap()
        d_rcp = nc.declare_dram_parameter("d_rcp", [1, 512], F32, True).ap()
        d_rb = nc.declare_dram_parameter("d_rb", [1, 512], F32, True).ap()

    with tile.TileContext(nc) as tc:
        const_cm, const = _pool(tc, "const", 1)
        work_cm, work = _pool(tc, "work", 8)       # f32 [128,512] scratch
        stat_cm, stat = _pool(tc, "stat", 4)

        # ---- constants ----
        vecs = const.tile([P, 6, DC], F32, tag="vecs")
        nc.sync.dma_start(vecs[:], vecs_d)
        b1t = const.tile([P, NHC], F32, tag="b1t")
        nc.sync.dma_start(b1t[:], b1t_d)
        bvrow = const.tile([1, D], F32, tag="bvrow")
        nc.sync.dma_start(bvrow[:], bvrow_d)
        bvb = const.tile([P, D], F32, tag="bvb")
        nc.gpsimd.partition_broadcast(bvb[:], bvrow[:])
        onesP = const.tile([P, 1], F32, tag="onesP")
        nc.vector.memset(onesP[:], 1.0)
        onesPr = const.tile([P, 1], F32R, tag="onesPr")
        nc.sync.dma_start(onesPr[:], onesr_d)
        onesPb = const.tile([P, 1], BF16, tag="onesPb")
        nc.vector.memset(onesPb[:], 1.0)
        eps1 = const.tile([1, 1], F32, tag="eps1")
        nc.vector.memset(eps1[:], EPS)

        def scratch(name):
            return work.tile([P, 512], F32, tag="scratch", name=name)

        def gcol(idx, dc):
            return vecs[:, idx, dc : dc + 1]

        def ln_alloc(out_pool, out_tag, nt, paired):
            if paired:
                return [out_pool.tile([P, 2, nt], FP8, tag=out_tag,
                                      name=f"{out_tag}{i}")
                        for i in range(DC // 2)]
            return [out_pool.tile([P, nt], BF16, tag=out_tag,
                                  name=f"{out_tag}{i}") for i in range(DC)]

        def ln_chunk(ps, load_fn, outs, tt, out_tag, src_r, paired):
            """One 512-token LayerNorm chunk (gamma/beta folded host-side)."""
            if True:
                sl = slice(tt * 512, (tt + 1) * 512)
                raw = [load_fn(dc, tt) for dc in range(DC)]
                srcs = [r.bitcast(F32) if src_r else r for r in raw]
                pr_row = ps.tile([P, 512], F32, tag="row", bufs=2,
                                 name=f"lnrow_{out_tag}_{tt}")
                # sum on partition 0, sumsq on partition 32 (same bank)
                for dc in range(DC):
                    sq = work.tile([P, 512], BF16, tag="sq", bufs=2,
                                   name=f"sq_{out_tag}_{tt}_{dc}")
                    nc.scalar.square(sq[:], srcs[dc])
                    nc.tensor.matmul(
                        pr_row[0:1, :],
                        onesPr[:] if src_r else onesPb[:],
                        raw[dc],
                        start=(dc == 0), stop=(dc == DC - 1),
                    )
                    nc.tensor.matmul(
                        pr_row[32:33, :], onesPb[:],
                        sq[:],
                        start=(dc == 0), stop=(dc == DC - 1),
                    )
                st = stat.tile([1, 3, 512], F32, tag="stats", bufs=2,
                               name=f"st_{out_tag}_{tt}")
                mu, var, rs = (st[:, i, :] for i in range(3))
                nc.vector.tensor_scalar_mul(mu, pr_row[0:1, :], 1.0 / D)
                nc.vector.tensor_scalar_mul(rs, pr_row[32:33, :], 1.0 / D)
                nc.vector.tensor_mul(var, mu, mu)
                nc.vector.tensor_sub(var, rs, var)
                # rs <- sqrt(var+eps) then var <- 1/rs (fast approx)
                nc.scalar.activation(rs, var, AF.Sqrt, bias=eps1[:])
                nc.vector.reciprocal_approx_fast(var, rs)
                rs = var
                # bf16 stats rows -> bf16 broadcasts -> bf16 2x normalize
                stb = stat.tile([1, 2, 512], BF16, tag="statsb", bufs=1,
                                name=f"stb_{out_tag}_{tt}")
                mu_b, rs_b = stb[:, 0, :], stb[:, 1, :]
                nc.vector.tensor_copy(mu_b, mu)
                nc.vector.tensor_copy(rs_b, rs)
                mub = work.tile([P, 512], BF16, tag="mub", bufs=2,
                                name=f"mub_{out_tag}_{tt}")
                nc.gpsimd.partition_broadcast(mub[:], mu_b)
                rsb = work.tile([P, 512], BF16, tag="rsb", bufs=2,
                                name=f"rsb_{out_tag}_{tt}")
                nc.gpsimd.partition_broadcast(rsb[:], rs_b)
                for dc in range(DC):
                    # ActE evicts src to bf16 so both DVE ops run 2x mode
                    xb = work.tile([P, 512], BF16, tag="xb", bufs=2,
                                   name=f"xb_{out_tag}_{tt}_{dc}")
                    nc.scalar.activation(xb[:], srcs[dc], AF.Copy)
                    t = work.tile([P, 512], BF16, tag="lnt", bufs=2,
                                  name=f"lnt_{out_tag}_{tt}_{dc}")
                    nc.vector.tensor_sub(t[:], xb[:], mub[:])
                    dst = (outs[dc // 2][:, dc % 2, sl] if paired
                           else outs[dc][:, sl])
                    nc.vector.tensor_mul(dst, t[:], rsb[:])

        def ln_T(ps, load_fn, nt, out_pool, out_tag, src_r, paired=False):
            outs = ln_alloc(out_pool, out_tag, nt, paired)
            for tt in range(nt // 512):
                ln_chunk(ps, load_fn, outs, tt, out_tag, src_r, paired)
            return outs

        def rope_evict(psum, out_ap, cos_t, sin_t, sl, b_idx, b_rot_idx, fc):
            """out = (psum + b)*cosP + pairswap(psum + b)*sinE (bf16).

            bR = pairswap(b), so applying b on the ActE eviction (bias is
            per-partition there, free) makes the shuffled copy carry bR
            automatically; both DVE multiplies then run as bf16 2x TT
            (STT has no 2x uop and costs ~745ns vs 345ns for TT).
            """
            pb = work.tile([P, 512], BF16, tag="ropePb", bufs=2,
                           name=f"ropeP_{b_idx}_{fc}_{sl.start}")
            nc.scalar.activation(pb[:], psum[:], AF.Identity,
                                 bias=gcol(b_idx, fc))
            sh = work.tile([P, 512], BF16, tag="ropeSh", bufs=2,
                           name=f"ropeS_{b_idx}_{fc}_{sl.start}")
            nc.vector.stream_shuffle(sh[:], pb[:], PAIRSWAP_MASK)
            t = work.tile([P, 512], BF16, tag="ropeT", bufs=2,
                          name=f"ropeA_{b_idx}_{fc}_{sl.start}")
            nc.vector.tensor_mul(t[:], pb[:], cos_t[:, sl])
            t2 = work.tile([P, 512], BF16, tag="ropeT", bufs=2,
                           name=f"ropeB_{b_idx}_{fc}_{sl.start}")
            nc.vector.tensor_mul(t2[:], sh[:], sin_t[:, sl])
            nc.vector.tensor_add(out_ap, t[:], t2[:])

        def dram_loader(pool, dram_ap, tag):
            def load(dc, tt):
                t = pool.tile([P, 512], F32R, tag=tag,
                              name=f"{tag}_{dc}_{tt}")
                nc.sync.dma_start(
                    t[:], dram_ap[dc * P : (dc + 1) * P,
                                  tt * 512 : (tt + 1) * 512])
                return t[:]
            return load

        ps1_cm, ps1 = _psum(tc, "ps1")

        def dbg_dump(src_ap, dram_ap, n):
            if not DBG:
                return
            for c in range(n // 512):
                s = work.tile([P, 512], F32, tag="dbgs", bufs=2,
                              name=f"dbg_{dram_ap}_{c}")
                nc.vector.tensor_copy(
                    s[:], src_ap[:, c * 512 : (c + 1) * 512])
                nc.sync.dma_start(
                    dram_ap[:, c * 512 : (c + 1) * 512], s[:])

        # ====== front: LN(ctx), then {LN(x) | k | v | q} interleaved ======
        # weights prefetched up front so DMA hides under LN compute
        w_cm, w_p = _pool(tc, "wqkv", 8, side="right")
        chat_cm, chat_p = _pool(tc, "chat", DC, side="right")
        # left-stack order chosen for LIFO exits:
        # qT/kT/vext (die after attention) below xhat (dies after q),
        # then cin (dies after ctx-LN), then xin (dies after interleave)
        qT_cm, qT_p = _pool(tc, "qT", DC)
        kT_cm, kT_p = _pool(tc, "kT", DC)
        vext_cm, vext_p = _pool(tc, "vext", KC)
        xhat_cm, xhat_p = _pool(tc, "xhat", DC)

        cin_cm, cin_p = _pool(tc, "cin", 12)
        chatT = ln_alloc(chat_p, "chat", TK, paired=True)
        cin_load = dram_loader(cin_p, ctxT, "cin")
        wk_sb = wv_sb = None
        for tt in range(TK // 512):
            ln_chunk(ps1, cin_load, chatT, tt, "chat", True, True)
            if tt == 0:
                # weight prefetch behind the first input chunk's DMAs
                wk_sb = w_p.tile([P, DC, D], FP8, tag="w8", bufs=2,
                                 name="wk8")
                nc.sync.dma_start(wk_sb[:], wk)
                wv_sb = w_p.tile([P, DC, D], FP8, tag="w8", bufs=2,
                                 name="wv8")
                nc.sync.dma_start(wv_sb[:], wv)
        cin_cm.__exit__(None, None, None)

        ropek_cm, ropek_p = _pool(tc, "ropek", 1, side="right")
        cosk_t = ropek_p.tile([P, TK], BF16, tag="cosk")
        nc.sync.dma_start(cosk_t[:], cosk)
        sink_t = ropek_p.tile([P, TK], BF16, tag="sink")
        nc.sync.dma_start(sink_t[:], sink)

        xin_cm, xin_p = _pool(tc, "xin", 8)
        xhatT = ln_alloc(xhat_p, "xhat", TQ, paired=True)
        xin_load = dram_loader(xin_p, xT, "xin")
        kT = [kT_p.tile([P, TK], BF16, tag="kT", name=f"kT{i}")
              for i in range(DC)]
        vext = []
        for kc in range(KC):
            vt = vext_p.tile([P, H, HD + 1], FP8, tag="vext",
                             name=f"vext{kc}")
            nc.vector.memset(vt[:, :, HD : HD + 1], 1.0)
            vext.append(vt)

        def k_unit(fc):
            for tt in range(TK // 512):
                sl = slice(tt * 512, (tt + 1) * 512)
                pm = ps1.tile([P, 512], F32, tag="mm", bufs=4,
                              name=f"pmk_{fc}_{tt}")
                for i in range(DC // 2):
                    nc.tensor.matmul(
                        pm[:],
                        wk_sb[:, 2 * i : 2 * i + 2, fc * P : (fc + 1) * P],
                        chatT[i][:, :, sl],
                        start=(i == 0), stop=(i == DC // 2 - 1),
                        perf_mode=DR,
                    )
                rope_evict(pm, kT[fc][:, sl], cosk_t, sink_t, sl,
                           V_BK, V_BKR, fc)

        def v_unit(kc):
            for f2 in range(2):
                pm = ps1.tile([P, 512], F32, tag="mm", bufs=4,
                              name=f"pmv_{kc}_{f2}")
                for i in range(DC // 2):
                    nc.tensor.matmul(
                        pm[:], chatT[i][:, :, kc * P : (kc + 1) * P],
                        wv_sb[:, 2 * i : 2 * i + 2,
                              f2 * 512 : (f2 + 1) * 512],
                        start=(i == 0), stop=(i == DC // 2 - 1),
                        perf_mode=DR,
                    )
                nc.vector.tensor_add(
                    vext[kc][:, f2 * 8 : (f2 + 1) * 8, 0:HD],
                    pm[:].rearrange("p (h d) -> p h d", d=HD),
                    bvb[:, f2 * 512 : (f2 + 1) * 512].rearrange(
                        "p (h d) -> p h d", d=HD),
                )

        def q_unit(fc):
            for tt in range(TQ // 512):
                sl = slice(tt * 512, (tt + 1) * 512)
                pm = ps1.tile([P, 512], F32, tag="mm", bufs=4,
                              name=f"pmq_{fc}_{tt}")
                for i in range(DC // 2):
                    nc.tensor.matmul(
                        pm[:],
                        wq_sb[:, 2 * i : 2 * i + 2, fc * P : (fc + 1) * P],
                        xhatT[i][:, :, sl],
                        start=(i == 0), stop=(i == DC // 2 - 1),
                        perf_mode=DR,
                    )
                rope_evict(pm, qT[fc][:, sl], cosq_t, sinq_t, sl,
                           V_BQ, V_BQR, fc)

        # interleave: PE-heavy k/v units hide LN(x)/rope-k DVE+ActE work
        for fc in range(DC):
            k_unit(fc)
            v_unit(2 * fc)
            v_unit(2 * fc + 1)
            if fc < TQ // 512:
                ln_chunk(ps1, xin_load, xhatT, fc, "xhat", True, True)
        xin_cm.__exit__(None, None, None)
        ropek_cm.__exit__(None, None, None)
        chat_cm.__exit__(None, None, None)

        # q last (xin/ropek space freed)
        wqp_cm, wqp_p = _pool(tc, "wqp", 1, side="right")
        wq_sb = wqp_p.tile([P, DC, D], FP8, tag="wq8", name="wq8")
        nc.sync.dma_start(wq_sb[:], wq)
        ropeq_cm, ropeq_p = _pool(tc, "ropeq", 1, side="right")
        cosq_t = ropeq_p.tile([P, TQ], BF16, tag="cosq")
        nc.sync.dma_start(cosq_t[:], cosq)
        sinq_t = ropeq_p.tile([P, TQ], BF16, tag="sinq")
        nc.sync.dma_start(sinq_t[:], sinq)
        qT = [qT_p.tile([P, TQ], BF16, tag="qT", name=f"qT{i}")
              for i in range(DC)]
        for fc in range(DC):
            q_unit(fc)
        ropeq_cm.__exit__(None, None, None)
        wqp_cm.__exit__(None, None, None)
        xhat_cm.__exit__(None, None, None)
        w_cm.__exit__(None, None, None)
        ps1_cm.__exit__(None, None, None)

        # ================= phase 5: attention =================
        # Per (tt, head-pair): 16 kc steps. Each kc: two concurrent K=64
        # score matmuls (PE row tiles 0/64) into a 2-bank psum pair-tile,
        # one [128,1024] exp, two U accumulations (M=65, ones-column
        # denominator). Head pairs processed two at a time so softmax
        # reciprocals batch 4 heads per DVE call.
        ps5_cm, ps5 = _psum(tc, "ps5")
        # wo prefetched here so its DMA hides under attention
        wo_cm, wo_p = _pool(tc, "wo", DC, side="right")
        wo_sb = []
        for dc in range(DC):
            wt = wo_p.tile([P, D], BF16, tag="wo", name=f"wo{dc}")
            nc.sync.dma_start(wt[:], wo[dc * P : (dc + 1) * P, :])
            wo_sb.append(wt)
        attnT_cm, attnT_p = _pool(tc, "attnT", DC, side="right")
        e_cm, e_p = _pool(tc, "epool", 4)
        attnT = [attnT_p.tile([P, TQ], BF16, tag="attnT", name=f"attnT{i}")
                 for i in range(DC)]
        for tt in range(TQ // 512):
            sl = slice(tt * 512, (tt + 1) * 512)
            for hp2 in range(4):       # pairs of head-pairs
                pus = []
                for i in range(2):     # head pair index within group
                    fc = hp2 * 2 + i
                    pu = ps5.tile([P, 2, 512], F32, tag="u", bufs=2,
                                  name=f"pu_{tt}_{fc}")
                    pus.append(pu)
                    for kc in range(KC):
                        psc = ps5.tile([P, 2, 512], F32, tag="sc", bufs=2,
                                       name=f"psc_{tt}_{fc}_{kc}")
                        for j in range(2):   # head row-halves, concurrent
                            hb = j * HD
                            nc.tensor.matmul(
                                psc[:, j, :],
                                kT[fc][hb : hb + HD, kc * P : (kc + 1) * P],
                                qT[fc][hb : hb + HD, sl],
                                start=True, stop=True,
                            )
                        e = e_p.tile([P, 2, 512], BF16, tag="e",
                                     name=f"e_{tt}_{fc}_{kc}")
                        nc.scalar.activation(e[:], psc[:], AF.Exp, scale=0.125)
                        for j in range(2):
                            nc.tensor.matmul(
                                pu[0 : HD + 1, j, :],
                                vext[kc][:, fc * 2 + j, :],
                                e[:, j, :],
                                start=(kc == 0), stop=(kc == KC - 1),
                            )
                # softmax epilogue: copy den row to a partition-0 SBUF tile
                # (the custom-DVE reciprocal only honors base_partition 0),
                # then one fast-approx reciprocal for both heads
                for i in range(2):
                    fc = hp2 * 2 + i
                    den0 = stat.tile([1, 2, 512], F32, tag="den0", bufs=1,
                                     name=f"den0_{tt}_{fc}")
                    nc.vector.tensor_copy(
                        den0[:], pus[i][HD : HD + 1, :, :])
                    rcp = stat.tile([1, 2, 512], F32, tag="rcp", bufs=2,
                                    name=f"rcp_{tt}_{fc}")
                    nc.vector.reciprocal_approx_fast(rcp[:], den0[:])
                    for j in range(2):
                        hb = j * HD
                        rb = work.tile([HD, 512], F32, tag="rb", bufs=2,
                                       name=f"rb_{tt}_{fc}_{j}")
                        nc.gpsimd.partition_broadcast(rb[:], rcp[:, j, :])
                        nc.vector.tensor_mul(
                            attnT[fc][hb : hb + HD, sl],
                            pus[i][0:HD, j, :], rb[:])
        if DBG:
            dbg_dump(attnT[0][:], d_attnT, TQ)
        e_cm.__exit__(None, None, None)
        vext_cm.__exit__(None, None, None)
        kT_cm.__exit__(None, None, None)
        qT_cm.__exit__(None, None, None)
        ps5_cm.__exit__(None, None, None)

        # ================= phase 6: x2^T = Wo^T attn + x^T + bo ========
        ps6_cm, ps6 = _psum(tc, "ps6")
        x2_cm, x2_p = _pool(tc, "x2", DC)
        xin6_cm, xin6_p = _pool(tc, "xin6", 4, side="right")
        xhat2_cm, xhat2_p = _pool(tc, "xhat2", DC, side="right")
        h1_cm, h1_p = _pool(tc, "h1", 2 * NHC)
        w1_cm, w1_p = _pool(tc, "w1s", 4)
        x2T = [x2_p.tile([P, TQ], BF16, tag="x2", name=f"x2T{i}")
               for i in range(DC)]
        xhat2T = ln_alloc(xhat2_p, "xhat2", TQ, False)
        h1 = {}

        def outproj_unit(tt, fc):
            sl = slice(tt * 512, (tt + 1) * 512)
            xres = xin6_p.tile([P, 512], F32R, tag="xin6",
                               name=f"xres_{fc}_{tt}")
            nc.sync.dma_start(xres[:], xT[fc * P : (fc + 1) * P, sl])
            pm = ps6.tile([P, 512], F32, tag="mm", bufs=4,
                          name=f"pmo_{fc}_{tt}")
            for dc in range(DC):
                nc.tensor.matmul(
                    pm[:], wo_sb[dc][:, fc * P : (fc + 1) * P],
                    attnT[dc][:, sl], start=(dc == 0), stop=(dc == DC - 1),
                )
            nc.vector.scalar_tensor_tensor(
                x2T[fc][:, sl], pm[:], gcol(V_BO, fc),
                xres[:].bitcast(F32),
                ALU.add, ALU.add,
            )

        def w1_unit(tt, hc):
            sl = slice(tt * 512, (tt + 1) * 512)
            w1t = w1_p.tile([P, DC, P], BF16, tag="w1",
                            name=f"w1_{tt}_{hc}")
            nc.sync.dma_start(w1t[:], w1[:, hc, :, :])
            ph = ps6.tile([P, 512], F32, tag="mm", bufs=4,
                          name=f"ph1_{tt}_{hc}")
            for dc in range(DC):
                nc.tensor.matmul(
                    ph[:], w1t[:, dc, :],
                    xhat2T[dc][:, sl],
                    start=(dc == 0), stop=(dc == DC - 1),
                )
            ht = h1_p.tile([P, 512], BF16, tag="h1", name=f"h1_{tt}_{hc}")
            nc.scalar.activation(ht[:], ph[:], AF.Gelu,
                                 bias=b1t[:, hc : hc + 1])
            h1[tt, hc] = ht

        x2_load = lambda dc, tt: x2T[dc][:, tt * 512 : (tt + 1) * 512]
        for fc in range(DC):
            outproj_unit(0, fc)
        ln_chunk(ps6, x2_load, xhat2T, 0, "xhat2", False, False)
        # W1(tt0) PE work hides outproj(tt1)/LN2(tt1) DVE+ActE work;
        # the last 6 units are held back to cover LN2(tt1)'s DVE chain
        HB = 6
        for fc in range(DC):
            outproj_unit(1, fc)
            for hc in range(4 * fc, 4 * fc + 4):
                if hc < NHC - HB:
                    w1_unit(0, hc)
        ln_chunk(ps6, x2_load, xhat2T, 1, "xhat2", False, False)
        for hc in range(NHC - HB, NHC):
            w1_unit(0, hc)
        for hc in range(NHC):
            w1_unit(1, hc)
        if DBG:
            dbg_dump(x2T[0][:], d_x2T, TQ)
        w1_cm.__exit__(None, None, None)
        xhat2_cm.__exit__(None, None, None)
        xin6_cm.__exit__(None, None, None)
        attnT_cm.__exit__(None, None, None)
        wo_cm.__exit__(None, None, None)
        ps6_cm.__exit__(None, None, None)

        # ================= phase 8b: MLP down-proj, 8-bank pass ========
        ps8_cm, ps8 = _psum(tc, "ps8")
        w2_cm, w2_p = _pool(tc, "w2s", 5)
        out_cm, out_p = _pool(tc, "ostage", 4)
        for tt in range(TQ // 512):
            sl = slice(tt * 512, (tt + 1) * 512)
            pmsA = ps8.tile([P, DC // 2, 512], F32, tag="mlp8a", bufs=1,
                            name=f"pmh2a_{tt}")
            pmsB = ps8.tile([P, DC // 2, 512], F32, tag="mlp8b", bufs=1,
                            name=f"pmh2b_{tt}")
            halves = (pmsA, pmsB)
            for hc in range(NHC):
                w2t = w2_p.tile([P, D], BF16, tag="w2",
                                name=f"w2_{tt}_{hc}")
                nc.sync.dma_start(w2t[:], w2[hc * P : (hc + 1) * P, :])
                for fc in range(DC):
                    nc.tensor.matmul(
                        halves[fc // 4][:, fc % 4, :],
                        w2t[:, fc * P : (fc + 1) * P],
                        h1[tt, hc][:], start=(hc == 0), stop=(hc == NHC - 1),
                    )
            for fc in range(DC):
                ot = out_p.tile([P, 512], F32, tag="ostage",
                                name=f"ot_{tt}_{fc}")
                nc.vector.scalar_tensor_tensor(
                    ot[:], halves[fc // 4][:, fc % 4, :],
                    gcol(V_B2, fc), x2T[fc][:, sl],
                    ALU.add, ALU.add,
                )
                nc.sync.dma_start(outT[fc * P : (fc + 1) * P, sl], ot[:])

        out_cm.__exit__(None, None, None)
        w2_cm.__exit__(None, None, None)
        h1_cm.__exit__(None, None, None)
        x2_cm.__exit__(None, None, None)
        ps8_cm.__exit__(None, None, None)
        stat_cm.__exit__(None, None, None)
        work_cm.__exit__(None, None, None)
        const_cm.__exit__(None, None, None)

    nc.compile()
    return nc


# old feature index (within a 64-dim head block) at each new position:
# pairs (j, j+32) become adjacent (2j, 2j+1)
OLD_OF_NEW = np.array([j // 2 if j % 2 == 0 else j // 2 + 32
                       for j in range(HD)])


def _perm_cols(a):
    """Permute the last dim (64-multiple) per 64-feature head block."""
    a = np.asarray(a, np.float32)
    shp = a.shape
    nb = shp[-1] // HD
    a = a.reshape(shp[:-1] + (nb, HD))
    a = a[..., OLD_OF_NEW]
    return a.reshape(shp)


def _pairswap(a):
    """Swap even/odd positions of the last dim."""
    a = np.asarray(a, np.float32)
    shp = a.shape
    a = a.reshape(shp[:-1] + (shp[-1] // 2, 2))
    a = a[..., ::-1]
    return np.ascontiguousarray(a.reshape(shp))


def _col8(v):
    return np.ascontiguousarray(
        np.asarray(v, np.float32).reshape(DC, P).T.astype(np.float32))


def make_in_maps(inputs):
    x = np.asarray(inputs["x"], np.float32)
    context = np.asarray(inputs["context"], np.float32)
    cos = np.asarray(inputs["rope_cos"], np.float32).reshape(L, HD)
    sin = np.asarray(inputs["rope_sin"], np.float32).reshape(L, HD)

    bf = lambda a: np.ascontiguousarray(np.asarray(a, np.float32)).astype(
        ml_dtypes.bfloat16)
    Wq = np.asarray(inputs["Wq"], np.float32)
    Wkv = np.asarray(inputs["Wkv"], np.float32)
    W1 = np.asarray(inputs["W1"], np.float32)
    g_q = np.asarray(inputs["g_q"], np.float32)
    be_q = np.asarray(inputs["be_q"], np.float32)
    g_kv = np.asarray(inputs["g_kv"], np.float32)
    be_kv = np.asarray(inputs["be_kv"], np.float32)
    g_ffn = np.asarray(inputs["g_ffn"], np.float32)
    be_ffn = np.asarray(inputs["be_ffn"], np.float32)

    # fold LN gamma/beta into weights/biases
    Wq_f = g_q[:, None] * Wq
    bq_f = be_q @ Wq + np.asarray(inputs["bq"], np.float32)
    Wk_f = g_kv[:, None] * Wkv[:, :D]
    bk_f = be_kv @ Wkv[:, :D] + np.asarray(inputs["bkv"], np.float32)[:D]
    Wv_f = g_kv[:, None] * Wkv[:, D:]
    bv_f = be_kv @ Wkv[:, D:] + np.asarray(inputs["bkv"], np.float32)[D:]
    W1_f = g_ffn[:, None] * W1
    b1_f = be_ffn @ W1 + np.asarray(inputs["b1"], np.float32)

    # rope pair permutation on q/k output features
    Wq_p = _perm_cols(Wq_f)
    bq_p = _perm_cols(bq_f)
    Wk_p = _perm_cols(Wk_f)
    bk_p = _perm_cols(bk_f)

    # q/k/v weights: fp8e4m3 packed [p, dc, m] = W[dc*128+p, m]
    f8 = lambda a: np.ascontiguousarray(
        np.asarray(a, np.float32).reshape(DC, P, D).transpose(1, 0, 2)
    ).astype(ml_dtypes.float8_e4m3fn)
    wq_b = f8(Wq_p)
    wk_b = f8(Wk_p)
    wv_b = f8(Wv_f)
    wo_b = bf(inputs["Wo"])
    # w1 packed [p, hc, dc, j] = W1[dc*128+p, hc*128+j]
    w1_b = bf(np.ascontiguousarray(
        W1_f.reshape(DC, P, NHC, P).transpose(1, 2, 0, 3)))
    w2_b = bf(inputs["W2"])

    vecs = np.stack(
        [_col8(bq_p), _col8(bk_p),
         _col8(inputs["bo"]), _col8(inputs["b2"]),
         _col8(_pairswap(bq_p)), _col8(_pairswap(bk_p))],
        axis=1,
    )  # [128, 6, 8]
    vecs = np.ascontiguousarray(vecs)
    b1t = np.ascontiguousarray(b1_f.reshape(NHC, P).T)
    bvrow = np.ascontiguousarray(bv_f.reshape(1, D))

    # rope tables in permuted feature space:
    # cosP[n] = cos[old_of_new[n]]; sinE[2j] = -sin[j], sinE[2j+1] = sin[j+32]
    cosP = cos[:, OLD_OF_NEW]                        # [L, 64]
    sinP = sin[:, OLD_OF_NEW]
    sinE = sinP.copy()
    sinE[:, 0::2] = -sinE[:, 0::2]
    cosT = cosP.T                                    # [64, L]
    sinT = sinE.T
    cosk_full = np.ascontiguousarray(np.concatenate([cosT, cosT], 0)).astype(
        ml_dtypes.bfloat16)
    sink_full = np.ascontiguousarray(np.concatenate([sinT, sinT], 0)).astype(
        ml_dtypes.bfloat16)

    in_maps = []
    for c in range(NCORES):
        b, hf = c // 2, c % 2
        tsl = slice(hf * TQ, (hf + 1) * TQ)
        in_maps.append({
            "xT": np.ascontiguousarray(x[b, tsl, :].T),
            "ctxT": np.ascontiguousarray(context[b].T),
            "cosq": np.ascontiguousarray(cosk_full[:, tsl]),
            "sinq": np.ascontiguousarray(sink_full[:, tsl]),
            "cosk": cosk_full,
            "sink": sink_full,
            "wq": wq_b, "wk": wk_b, "wv": wv_b, "wo": wo_b,
            "w1": w1_b, "w2": w2_b,
            "vecs": vecs, "b1t": b1t, "bvrow": bvrow,
            "onesr": np.ones((P, 1), np.float32),
        })
    return in_maps


def kernel(**inputs) -> np.ndarray:
    global _CACHED_NC
    if _CACHED_NC is None:
        _CACHED_NC = build_nc()
    nc = _CACHED_NC
    in_maps = make_in_maps(inputs)
    res = run_bass_kernel_spmd(nc, in_maps, core_ids=list(range(NCORES)))
    out = np.empty((B, L, D), np.float32)
    for c in range(NCORES):
        b, hf = c // 2, c % 2
        out[b, hf * TQ : (hf + 1) * TQ, :] = res.results[c]["outT"].T
    return out

